# revision 17
# baseline (speedup 1.0000x reference)
"""Deformable 3D convolution (DeformConv3d) on 8 TRN2 NeuronCores via Bass/Tile.

Strategy (data-parallel over the 16 (b, z) output planes, 2 per core):
  - Host packs x into a zero-padded bf16 "quad image": for every padded pixel
    (dp, hp, wp) a 128-element row [t=(cy,j) major, c minor] holding the
    2x2 bilinear corner patch across all 32 channels.  One dma_gather
    descriptor (256B) fetches all 4 corners x 32 channels for one
    (tap, sample) pair.
  - Device, per core: field phase computes floor/frac/corner weights for all
    18 chunks in a handful of large DVE ops; an upfront wrap phase turns px
    (split hi*128+lo so the selection matmuls run in bf16, recombined by PSUM
    accumulation) into the int16 gather-index layout, materialized directly
    in all SWDGE queue idx bands by banded selection matmuls; per 128-sample
    chunk dma_gather (8 calls round-robin over the 4 queues, queue loads
    alternated per chunk for balance) lands G[s, (pl,k), (t,c)] bf16; the
    corner weights are c-expanded on the ACT engine so the DVE multiply gets
    two contiguous bf16 operands, pairwise adds sum the 4 corners; one XBAR
    DMA-transpose per 4-chunk group flips both planes' weighted sums into
    [kc, s] layout (no PE data transposes); the conv is 7 accumulating bf16
    matmuls of 512 columns per (plane, group), then bias-add and store.

  Bottleneck note: steady state is paced by SWDGE descriptor generation
    (~8.5 ns/descriptor per queue Q7 core, ~0.76 us/call, 4 queues, 1024
    idx/call hardware cap) at ~15-16 us per 6912-descriptor chunk; compute,
    DMA transfer and the conv are all overlapped underneath it.
"""

import numpy as np
import ml_dtypes

import concourse.bass as bass
import concourse.bacc as bacc
import concourse.mybir as mybir
from concourse import tile
from concourse import library_config
from concourse.bass_utils import run_bass_kernel_spmd
from concourse.tile_rust import add_dep_helper

F32 = mybir.dt.float32
BF16 = mybir.dt.bfloat16
I32 = mybir.dt.int32
I16 = mybir.dt.int16
AT = mybir.AluOpType
AF = mybir.ActivationFunctionType
AX = mybir.AxisListType

# problem constants
B, CIN, D, H, W = 2, 32, 8, 48, 48
K, COUT = 27, 64
S = H * W                      # 2304 samples per plane
DP, HPAD, WPAD = 10, 52, 52    # padded depth/rows/cols
PLANE_PX = DP * HPAD * WPAD    # 27040 quad rows per batch
ROW = 128                      # quad row payload elems (4 corners x 32 ch)
NCHUNK = S // 128              # 18
NCOL = 2 * K                   # 54 = (plane, tap) columns per chunk
# dma_gather call splits (<=1024 idx each).  Queue 0's descriptor
# generation runs synchronously ON the Pool engine (~10.6 ns/idx of engine
# residency, observed on HW); queues 1-3 hand off asynchronously (~600 ns
# dispatch) and generate in the background.  So: queues 1-3 carry most of
# the load (dispatched first), queue 0 a small tail share (dispatched
# last, so its engine-blocking overlaps the async queues' background
# generation).
CALL_SCHED = [(7, 1), (7, 2), (7, 3), (6, 0), (7, 1), (7, 2), (7, 3), (6, 0)]
CALL_COLS = [c for c, _ in CALL_SCHED]
N_CORES = 8
NQ = 4

_CACHE = {}
GATHER_DT = BF16               # quad image + G dtype


def build_nc(skip=(), debug=False):
    nc = bacc.Bacc("TRN2", target_bir_lowering=False, debug=False,
                   num_swdge_queues=NQ)
    xq = nc.dram_tensor("xq", [PLANE_PX, ROW], GATHER_DT, kind="ExternalInput")
    offs = nc.dram_tensor("offs", [S, 108], F32, kind="ExternalInput")
    msk = nc.dram_tensor("msk", [S, 54], F32, kind="ExternalInput")
    bases = nc.dram_tensor("bases", [S, 108], F32, kind="ExternalInput")
    dpk = nc.dram_tensor("dpk", [S, 54], F32, kind="ExternalInput")
    wt = nc.dram_tensor("wt", [128, 7 * 64], BF16, kind="ExternalInput")
    bia = nc.dram_tensor("bia", [64, 1], F32, kind="ExternalInput")
    idf = nc.dram_tensor("idf", [128, 16 * 128], BF16, kind="ExternalInput")
    out = nc.dram_tensor("out", [2, 64, S], F32, kind="ExternalOutput")
    if debug:
        dbg_px = nc.dram_tensor("dbg_px", [128, 54], F32, kind="ExternalOutput")
        dbg_wr = nc.dram_tensor("dbg_wr", [16, 432], I16, kind="ExternalOutput")
        dbg_wf = nc.dram_tensor("dbg_wf", [128, 4, 54], BF16, kind="ExternalOutput")

    with tile.TileContext(nc) as tc:
        with (
            tc.tile_pool(name="const", bufs=1) as pc,
            tc.tile_pool(name="fldT", bufs=1) as pt,   # transient field tensors
            tc.tile_pool(name="fldP", bufs=1) as pf,   # persistent px / wf
            tc.tile_pool(name="gg", bufs=3) as pg,
            tc.tile_pool(name="v4", bufs=2) as pv,
            tc.tile_pool(name="vs", bufs=2) as pvs,
            tc.tile_pool(name="vt", bufs=2) as ptt,
            tc.tile_pool(name="oo", bufs=2) as po,
            tc.tile_pool(name="psW", bufs=6, space="PSUM") as psW,
            tc.tile_pool(name="psC", bufs=2, space="PSUM") as psC,
        ):
            wt_t = pc.tile([128, 7 * 64], BF16)
            nc.sync.dma_start(wt_t[:], wt[:])
            bia_t = pc.tile([64, 1], F32)
            nc.sync.dma_start(bia_t[:], bia[:])
            idf_t = pc.tile([128, 16 * 128], BF16)
            nc.sync.dma_start(idf_t[:], idf[:])
            lib_inst = nc.gpsimd.load_library(library_config.mlp)

            # ---- input loads + field phase, in two segments so chunk 0's
            # gathers can dispatch long before the full field phase ends.
            offs_t = pt.tile([128, NCHUNK, 108], F32, tag="offs")
            bases_t = pt.tile([128, NCHUNK, 108], F32, tag="bases")
            msk_t = pt.tile([128, NCHUNK, 54], F32, tag="msk")
            dpk_t = pt.tile([128, NCHUNK, 54], F32, tag="dpk")
            hw_ = pt.tile([128, NCHUNK, 108], F32, tag="hw")
            ti_ = pt.tile([128, NCHUNK, 108], I32, tag="offs", name="ti_")
            tf_ = pt.tile([128, NCHUNK, 108], F32, tag="bases", name="tf_")
            gt_ = pt.tile([128, NCHUNK, 108], F32, tag="gt")
            px_ = pt.tile([128, NCHUNK, 54], F32, tag="bm", name="px_")
            ph_i = pt.tile([128, NCHUNK, 54], I32, tag="offs", name="ph_i")
            hi_i = pt.tile([128, NCHUNK, 54], I32, tag="gt", name="hi_i")
            lo_i = pt.tile([128, NCHUNK, 54], I32, tag="dpk", name="lo_i")
            pxhl_ = pf.tile([128, NCHUNK, 108], BF16, tag="pxhl")
            l_ = pt.tile([128, NCHUNK, 108], F32, tag="gt", name="l_")
            l1_ = pt.tile([128, NCHUNK, 108], F32, tag="hw", name="l1_")
            am_ = pt.tile([128, NCHUNK, 54], F32, tag="dpk", name="am_")
            bm_ = pt.tile([128, NCHUNK, 54], F32, tag="bm")
            wf_ = pf.tile([128, NCHUNK, 54, 4], BF16, tag="wf")
            wrd_all = pf.tile([128, NCHUNK, 432], I16, tag="wrd")

            def load_seg(c0, c1):
                s0, s1 = c0 * 128, c1 * 128
                nch = c1 - c0
                nc.sync.dma_start(
                    offs_t[:, c0:c1],
                    offs[s0:s1].rearrange("(c p) f -> p c f", p=128))
                nc.scalar.dma_start(
                    bases_t[:, c0:c1],
                    bases[s0:s1].rearrange("(c p) f -> p c f", p=128))
                nc.scalar.dma_start(
                    msk_t[:, c0:c1],
                    msk[s0:s1].rearrange("(c p) f -> p c f", p=128))
                nc.sync.dma_start(
                    dpk_t[:, c0:c1],
                    dpk[s0:s1].rearrange("(c p) f -> p c f", p=128))

            def field_seg(c0, c1):
                c = slice(c0, c1)
                nc.vector.tensor_tensor(out=hw_[:, c], in0=offs_t[:, c],
                                        in1=bases_t[:, c], op=AT.add)
                nc.vector.tensor_scalar(out=hw_[:, c], in0=hw_[:, c], scalar1=49.0,
                                        scalar2=0.0, op0=AT.min, op1=AT.max)
                nc.vector.tensor_copy(out=ti_[:, c], in_=hw_[:, c])
                nc.scalar.activation(out=tf_[:, c], in_=ti_[:, c], func=AF.Copy)
                nc.vector.tensor_tensor(out=gt_[:, c], in0=tf_[:, c],
                                        in1=hw_[:, c], op=AT.is_gt)
                nc.vector.tensor_tensor(out=tf_[:, c], in0=tf_[:, c],
                                        in1=gt_[:, c], op=AT.subtract)

                # px = floor_h * 52 + floor_w + dpk  (exact small ints in f32)
                nc.vector.tensor_scalar(out=px_[:, c], in0=tf_[:, c, :54],
                                        scalar1=52.0, scalar2=None, op0=AT.mult)
                nc.vector.tensor_tensor(out=px_[:, c], in0=px_[:, c],
                                        in1=tf_[:, c, 54:], op=AT.add)
                nc.vector.tensor_tensor(out=px_[:, c], in0=px_[:, c],
                                        in1=dpk_t[:, c], op=AT.add)

                # split px = hi*128 + lo so the wrap matmuls can run in bf16
                # (hi <= 211 and lo < 128 are bf16-exact).  px is an exact
                # integer in f32, so the i32 conversion is exact and hi/lo
                # are just a shift and a mask.
                nc.vector.tensor_copy(out=ph_i[:, c], in_=px_[:, c])
                nc.vector.tensor_scalar(out=hi_i[:, c], in0=ph_i[:, c], scalar1=7,
                                        scalar2=None, op0=AT.arith_shift_right)
                nc.vector.tensor_scalar(out=lo_i[:, c], in0=ph_i[:, c], scalar1=127,
                                        scalar2=None, op0=AT.bitwise_and)
                nc.vector.tensor_copy(out=pxhl_[:, c, :54], in_=hi_i[:, c])
                nc.vector.tensor_copy(out=pxhl_[:, c, 54:], in_=lo_i[:, c])

                nc.vector.tensor_tensor(out=l_[:, c], in0=hw_[:, c],
                                        in1=tf_[:, c], op=AT.subtract)
                nc.scalar.activation(out=l1_[:, c], in_=l_[:, c], func=AF.Copy,
                                     scale=-1.0, bias=1.0)

                # corner weights, col-major: wf[p, ci, (pl,k), t] bf16
                nc.vector.tensor_tensor(out=am_[:, c], in0=l1_[:, c, :54],
                                        in1=msk_t[:, c], op=AT.mult)
                nc.vector.tensor_tensor(out=bm_[:, c], in0=l_[:, c, :54],
                                        in1=msk_t[:, c], op=AT.mult)
                for t, (ab, lw0) in enumerate([(am_, l1_), (am_, l_),
                                               (bm_, l1_), (bm_, l_)]):
                    nc.vector.tensor_tensor(out=wf_[:, c, :, t], in0=ab[:, c],
                                            in1=lw0[:, c, 54:], op=AT.mult)

            # ---- wrap: one chunk's px into the dma_gather int16 index
            # layout, materialized directly in all 5 idx bands (parts 0:16 +
            # queue bands 16:32, 48:64, 80:96, 112:128): the banded
            # selection matmuls write wrp[band0+r, q*54+col] = px[q*16+r,
            # col]; one 128-partition DVE copy converts to i16 in the
            # wrapped (col*8+q) order.
            def wrap_chunk(ci):
                wrp = psW.tile([128, 432], F32, tag="wrap", space="PSUM",
                               name=f"wrp_{ci}")
                for q in range(8):
                    # PSUM-accumulated recombine: 128*hi (scaled selection
                    # matrix, cols q*128..) + lo (plain selection, cols
                    # (8+q)*128..)
                    nc.tensor.matmul(out=wrp[:, q * 54:(q + 1) * 54],
                                     lhsT=idf_t[:, q * 128:(q + 1) * 128],
                                     rhs=pxhl_[:, ci, :54], start=True, stop=False)
                    nc.tensor.matmul(out=wrp[:, q * 54:(q + 1) * 54],
                                     lhsT=idf_t[:, (8 + q) * 128:(9 + q) * 128],
                                     rhs=pxhl_[:, ci, 54:], start=False, stop=True)
                nc.vector.tensor_copy(
                    out=wrd_all[:, ci].rearrange("p (col q) -> p q col", q=8),
                    in_=wrp[:].rearrange("p (q col) -> p q col", col=54))

            # segment A: chunks 0-1 ready ASAP; wrap them; the rest of the
            # field phase is emitted just-in-time inside the chunk loop so
            # the scheduler can't starve the early critical chain with it.
            SEGA = 2
            load_seg(0, SEGA)
            field_seg(0, SEGA)
            for ci in range(SEGA):
                wrap_chunk(ci)
            load_seg(SEGA, NCHUNK)
            field_seg(SEGA, NCHUNK)
            if debug:
                nc.sync.dma_start(dbg_px[:], px_[:, 0, :])
                nc.sync.dma_start(dbg_wr[:], wrd_all[16:32, 0, :])
                nc.sync.dma_start(dbg_wf[:], wf_[:, 0, :, :])

            vs_cur = {}
            NG = 2                                     # chunks per conv group
            for ci in range(NCHUNK):
                gi0 = ci % NG == 0                     # group leader chunk
                ng = NG
                c0 = ci - ci % NG

                # wrap lookahead: keep the PE two chunks ahead of the gathers
                if SEGA <= ci + SEGA < NCHUNK:
                    wrap_chunk(ci + SEGA)

                # ---- gather: one 256B bf16 quad row per (plane, tap, sample)
                G = pg.tile([128, NCOL, ROW], GATHER_DT, tag="G")
                col0 = 0
                if "gather" in skip:
                    nc.vector.memset(G[:, :1, :1], 0)
                for ncols, qn in (CALL_SCHED if "gather" not in skip else []):
                    nidx = ncols * 128
                    gi = nc.gpsimd.dma_gather(
                        G[:, col0:col0 + ncols, :],
                        xq[:],
                        wrd_all[:, ci, col0 * 8: col0 * 8 + nidx // 16],
                        nidx, nidx, ROW, queue_num=qn,
                        single_packet=False)
                    add_dep_helper(gi.ins, lib_inst.ins, sync=False,
                                   reason="mlp library before dma_gather")
                    col0 += ncols

                # ---- corner-weight multiply + corner sum (DVE)
                if gi0:
                    vs_cur[0] = pvs.tile([128, 2, NG, 896], BF16, tag="vs",
                                         name=f"vs_{ci}")
                    nc.vector.memset(vs_cur[0][:, :, :, 864:], 0)
                if "vmul" not in skip:
                    # expand corner weights over c on the (idle) ACT engine so
                    # the DVE multiply gets two contiguous operands; per plane
                    # to halve the buffer
                    V4g = pv.tile([128, NCOL, 4, 32], BF16, tag="v4g", bufs=1)
                    for pl in range(2):
                        sl = slice(pl * K, (pl + 1) * K)
                        wfx = pv.tile([128, K, 4, 32], BF16, tag="wfx",
                                      name=f"wfx_{ci}_{pl}")
                        nc.scalar.activation(
                            out=wfx[:], in_=wf_[:, ci, sl].to_broadcast([128, K, 4, 32]),
                            func=AF.Copy)
                        nc.vector.tensor_tensor(
                            out=V4g[:, sl],
                            in0=G[:, sl].rearrange("p col (t c) -> p col t c", c=32),
                            in1=wfx[:],
                            op=AT.mult)
                    # pairwise corner sums: (t0+t1) + (t2+t3)
                    with nc.allow_low_precision("4-term bf16 corner sum"):
                        t01 = pv.tile([128, NCOL, 32], BF16, tag="t01", bufs=1)
                        nc.vector.tensor_tensor(out=t01[:], in0=V4g[:, :, 0, :],
                                                in1=V4g[:, :, 1, :], op=AT.add)
                        t23 = pv.tile([128, NCOL, 32], BF16, tag="t23", bufs=1)
                        nc.vector.tensor_tensor(out=t23[:], in0=V4g[:, :, 2, :],
                                                in1=V4g[:, :, 3, :], op=AT.add)
                        for pl in range(2):
                            sl = slice(pl * K, (pl + 1) * K)
                            nc.vector.tensor_tensor(
                                out=vs_cur[0][:, pl, ci - c0, :864],
                                in0=t01[:, sl], in1=t23[:, sl], op=AT.add)

                # ---- group end: XBAR transpose + conv matmuls
                if ci - c0 == ng - 1 and "conv" not in skip:
                    r0g = c0 * 128
                    Vs = vs_cur[0]
                    VtT = ptt.tile([128, 2 * NG * 7, 128], BF16, tag="vt",
                                   name=f"vt_{ci}")
                    nc.sync.dma_start_transpose(
                        VtT[:], Vs[:].rearrange("p a b c -> p (a b c)"))
                    rhs4 = VtT[:].rearrange("p (pl c4 g) s -> p pl g c4 s", pl=2, g=7)
                    for pl in range(2):
                        cp = psC.tile([64, NG * 128], F32, tag="conv", space="PSUM",
                                      name=f"cp_{ci}_{pl}")
                        for g in range(7):
                            nc.tensor.matmul(out=cp[:, :ng * 128],
                                             lhsT=wt_t[:, g * 64:(g + 1) * 64],
                                             rhs=rhs4[:, pl, g, :ng],
                                             start=(g == 0), stop=(g == 6))
                        ou = po.tile([64, NG * 128], F32, tag="ou",
                                     name=f"ou_{ci}_{pl}")
                        nc.vector.tensor_scalar(out=ou[:, :ng * 128], in0=cp[:, :ng * 128],
                                                scalar1=bia_t[:64, :],
                                                scalar2=None, op0=AT.add)
                        nc.scalar.dma_start(out[pl, :, r0g:r0g + ng * 128], ou[:, :ng * 128])

    nc.compile()
    return nc


def _prep_static():
    """Input-independent constant tensors."""
    yy, xx = np.meshgrid(np.arange(H), np.arange(W), indexing="ij")
    yy = yy.reshape(-1).astype(np.float32)
    xx = xx.reshape(-1).astype(np.float32)
    kd = (np.arange(K) // 9).astype(np.float32)
    kh = ((np.arange(K) // 3) % 3).astype(np.float32)
    kw = (np.arange(K) % 3).astype(np.float32)

    bases = np.zeros((S, 108), np.float32)
    for pl in range(2):
        bases[:, pl * K:(pl + 1) * K] = yy[:, None] + kh[None, :]
        bases[:, 54 + pl * K:54 + (pl + 1) * K] = xx[:, None] + kw[None, :]

    # banded wrap selection: idf[s, q*128 + band0 + r] = w for s = q*16 + r,
    # bands at partitions {0, 16, 48, 80, 112} (CoreSim + 4 SWDGE queues).
    # Blocks 0..7 carry weight 128 (px hi part), blocks 8..15 weight 1 (lo).
    idf = np.zeros((128, 16, 128), np.float32)
    for q in range(8):
        for r in range(16):
            for band0 in (0, 16, 48, 80, 112):
                idf[q * 16 + r, q, band0 + r] = 128.0
                idf[q * 16 + r, 8 + q, band0 + r] = 1.0
    idf = idf.reshape(128, 16 * 128).astype(ml_dtypes.bfloat16)
    return bases, kd, idf


def _prep_weights(weight, bias):
    # wt rows kc = k*32 + c ; wt[kc, o] = weight[o, c, k]
    wk = weight.reshape(COUT, CIN, K)          # [o, c, k]
    wt = np.zeros((896, COUT), np.float32)
    wt[:864] = wk.transpose(2, 1, 0).reshape(864, COUT)   # [k, c, o] -> rows k*32+c
    # pack [7, 128, 64] -> [128, 7*64] for a single contiguous DMA
    wt = wt.reshape(7, 128, COUT).transpose(1, 0, 2).reshape(128, 7 * COUT)
    wt = np.ascontiguousarray(wt).astype(ml_dtypes.bfloat16)
    bia = bias.reshape(64, 1).astype(np.float32)
    return wt, bia


def _prep_quad(x):
    """x [B, C, D, H, W] -> quad [B, PLANE_PX, 128] bfloat16."""
    xp = np.zeros((B, DP, HPAD + 1, WPAD + 1, CIN), np.float32)
    xp[:, 1:1 + D, 1:1 + H, 1:1 + W, :] = x.transpose(0, 2, 3, 4, 1)
    q = np.empty((B, DP, HPAD, WPAD, 4, CIN), np.float32)
    for t, (cy, j) in enumerate([(0, 0), (0, 1), (1, 0), (1, 1)]):
        q[..., t, :] = xp[:, :, cy:cy + HPAD, j:j + WPAD, :]
    q = q.reshape(B, PLANE_PX, ROW)
    if GATHER_DT == BF16:
        q = q.astype(ml_dtypes.bfloat16)
    return q


def make_in_maps(input, offset, mask, weight, bias):
    if "static" not in _CACHE:
        _CACHE["static"] = _prep_static()
    bases, kd, idf = _CACHE["static"]
    wt, bia = _prep_weights(weight, bias)
    quad = _prep_quad(input)

    offr = offset.reshape(B, K, 2, D, S)   # [b, k, comp, z, s]
    mr = mask.reshape(B, K, D, S)

    in_maps = []
    for core in range(N_CORES):
        bidx = core // 4
        z0 = (2 * core) % 8
        offs_c = np.empty((S, 108), np.float32)
        msk_c = np.empty((S, 54), np.float32)
        dpk_c = np.empty((S, 54), np.float32)
        for pl, z in enumerate((z0, z0 + 1)):
            offs_c[:, pl * K:(pl + 1) * K] = offr[bidx, :, 0, z, :].T
            offs_c[:, 54 + pl * K:54 + (pl + 1) * K] = offr[bidx, :, 1, z, :].T
            msk_c[:, pl * K:(pl + 1) * K] = mr[bidx, :, z, :].T
            dpk_c[:, pl * K:(pl + 1) * K] = ((z + kd) * (HPAD * WPAD))[None, :]
        in_maps.append({
            "xq": quad[bidx],
            "offs": offs_c,
            "msk": msk_c,
            "bases": bases,
            "dpk": dpk_c,
            "wt": wt,
            "bia": bia,
            "idf": idf,
        })
    return in_maps


def kernel(input, offset, mask, weight, bias):
    input = np.ascontiguousarray(input, np.float32)
    offset = np.ascontiguousarray(offset, np.float32)
    mask = np.ascontiguousarray(mask, np.float32)
    weight = np.ascontiguousarray(weight, np.float32)
    bias = np.ascontiguousarray(bias, np.float32)

    if "nc" not in _CACHE:
        _CACHE["nc"] = build_nc()
    nc = _CACHE["nc"]
    in_maps = make_in_maps(input, offset, mask, weight, bias)

    res = run_bass_kernel_spmd(nc, in_maps, core_ids=list(range(N_CORES)))

    out = np.empty((B, COUT, D, H, W), np.float32)
    for core in range(N_CORES):
        bidx = core // 4
        z0 = (2 * core) % 8
        o = np.asarray(res.results[core]["out"], np.float32)   # [2, 64, S]
        out[bidx, :, z0] = o[0].reshape(COUT, H, W)
        out[bidx, :, z0 + 1] = o[1].reshape(COUT, H, W)
    return out



# revision 20
# speedup vs baseline: 1.0408x; 1.0408x over previous
"""Deformable 3D convolution (DeformConv3d) on 8 TRN2 NeuronCores via Bass/Tile.

Strategy (data-parallel over the 16 (b, z) output planes, 2 per core):
  - Host packs x into a zero-padded bf16 "quad image": for every padded pixel
    (dp, hp, wp) a 128-element row [t=(cy,j) major, c minor] holding the
    2x2 bilinear corner patch across all 32 channels.  One dma_gather
    descriptor (256B) fetches all 4 corners x 32 channels for one
    (tap, sample) pair.
  - Device, per core: field phase computes floor/frac/corner weights for all
    18 chunks in a handful of large DVE ops; an upfront wrap phase turns px
    (split hi*128+lo so the selection matmuls run in bf16, recombined by PSUM
    accumulation) into the int16 gather-index layout, materialized directly
    in all SWDGE queue idx bands by banded selection matmuls; per 128-sample
    chunk dma_gather (8 calls round-robin over the 4 queues, queue loads
    alternated per chunk for balance) lands G[s, (pl,k), (t,c)] bf16; the
    corner weights are c-expanded on the ACT engine so the DVE multiply gets
    two contiguous bf16 operands, pairwise adds sum the 4 corners; one XBAR
    DMA-transpose per 4-chunk group flips both planes' weighted sums into
    [kc, s] layout (no PE data transposes); the conv is 7 accumulating bf16
    matmuls of 512 columns per (plane, group), then bias-add and store.

  Bottleneck note: steady state is paced by SWDGE descriptor generation
    (~8.5 ns/descriptor per queue Q7 core, ~0.76 us/call, 4 queues, 1024
    idx/call hardware cap) at ~15-16 us per 6912-descriptor chunk; compute,
    DMA transfer and the conv are all overlapped underneath it.
"""

import numpy as np
import ml_dtypes

import concourse.bass as bass
import concourse.bacc as bacc
import concourse.mybir as mybir
from concourse import tile
from concourse import library_config
from concourse.bass_utils import run_bass_kernel_spmd
from concourse.tile_rust import add_dep_helper

F32 = mybir.dt.float32
BF16 = mybir.dt.bfloat16
I32 = mybir.dt.int32
I16 = mybir.dt.int16
AT = mybir.AluOpType
AF = mybir.ActivationFunctionType
AX = mybir.AxisListType

# problem constants
B, CIN, D, H, W = 2, 32, 8, 48, 48
K, COUT = 27, 64
S = H * W                      # 2304 samples per plane
DP, HPAD, WPAD = 10, 52, 52    # padded depth/rows/cols
PLANE_PX = DP * HPAD * WPAD    # 27040 quad rows per batch
ROW = 128                      # quad row payload elems (4 corners x 32 ch)
NCHUNK = S // 128              # 18
NCOL = 2 * K                   # 54 = (plane, tap) columns per chunk
# dma_gather call splits (<=1024 idx each).  Queue 0's descriptor
# generation runs synchronously ON the Pool engine (~10.6 ns/idx of engine
# residency, observed on HW); queues 1-3 hand off asynchronously (~600 ns
# dispatch) and generate in the background.  So: queues 1-3 carry most of
# the load (dispatched first), queue 0 a small tail share (dispatched
# last, so its engine-blocking overlaps the async queues' background
# generation).
CALL_SCHED = [(7, 1), (7, 2), (7, 3), (6, 0), (7, 1), (7, 2), (7, 3), (6, 0)]
CALL_COLS = [c for c, _ in CALL_SCHED]
N_CORES = 8
NQ = 4

_CACHE = {}
GATHER_DT = BF16               # quad image + G dtype


def build_nc(skip=(), debug=False):
    nc = bacc.Bacc("TRN2", target_bir_lowering=False, debug=False,
                   num_swdge_queues=NQ)
    xq = nc.dram_tensor("xq", [PLANE_PX, ROW], GATHER_DT, kind="ExternalInput")
    offs = nc.dram_tensor("offs", [S, 108], F32, kind="ExternalInput")
    msk = nc.dram_tensor("msk", [S, 54], F32, kind="ExternalInput")
    bases = nc.dram_tensor("bases", [S, 108], F32, kind="ExternalInput")
    dpk = nc.dram_tensor("dpk", [S, 54], F32, kind="ExternalInput")
    wt = nc.dram_tensor("wt", [128, 7 * 64], BF16, kind="ExternalInput")
    bia = nc.dram_tensor("bia", [64, 1], F32, kind="ExternalInput")
    idf = nc.dram_tensor("idf", [128, 16 * 128], BF16, kind="ExternalInput")
    out = nc.dram_tensor("out", [2, 64, S], F32, kind="ExternalOutput")
    if debug:
        dbg_px = nc.dram_tensor("dbg_px", [128, 54], F32, kind="ExternalOutput")
        dbg_wr = nc.dram_tensor("dbg_wr", [16, 432], I16, kind="ExternalOutput")
        dbg_wf = nc.dram_tensor("dbg_wf", [128, 4, 54], BF16, kind="ExternalOutput")

    with tile.TileContext(nc) as tc:
        with (
            tc.tile_pool(name="const", bufs=1) as pc,
            tc.tile_pool(name="fldT", bufs=1) as pt,   # transient field tensors
            tc.tile_pool(name="fldP", bufs=1) as pf,   # persistent px / wf
            tc.tile_pool(name="gg", bufs=2) as pg,
            tc.tile_pool(name="v4", bufs=2) as pv,
            tc.tile_pool(name="vs", bufs=3) as pvs,
            tc.tile_pool(name="vt", bufs=2) as ptt,
            tc.tile_pool(name="oo", bufs=1) as po,
            tc.tile_pool(name="psW", bufs=6, space="PSUM") as psW,
            tc.tile_pool(name="psC", bufs=2, space="PSUM") as psC,
        ):
            wt_t = pc.tile([128, 7 * 64], BF16)
            nc.sync.dma_start(wt_t[:], wt[:])
            bia_t = pc.tile([64, 1], F32)
            nc.sync.dma_start(bia_t[:], bia[:])
            idf_t = pc.tile([128, 16 * 128], BF16)
            nc.sync.dma_start(idf_t[:], idf[:])
            lib_inst = nc.gpsimd.load_library(library_config.mlp)

            # ---- input loads + field phase, in two segments so chunk 0's
            # gathers can dispatch long before the full field phase ends.
            offs_t = pt.tile([128, NCHUNK, 108], F32, tag="offs")
            bases_t = pt.tile([128, NCHUNK, 108], F32, tag="bases")
            msk_t = pt.tile([128, NCHUNK, 54], F32, tag="msk")
            dpk_t = pt.tile([128, NCHUNK, 54], F32, tag="dpk")
            hw_ = pt.tile([128, NCHUNK, 108], F32, tag="hw")
            ti_ = pt.tile([128, NCHUNK, 108], I32, tag="offs", name="ti_")
            tf_ = pt.tile([128, NCHUNK, 108], F32, tag="bases", name="tf_")
            gt_ = pt.tile([128, NCHUNK, 108], F32, tag="gt")
            px_ = pt.tile([128, NCHUNK, 54], F32, tag="bm", name="px_")
            ph_i = pt.tile([128, NCHUNK, 54], I32, tag="offs", name="ph_i")
            hi_i = pt.tile([128, NCHUNK, 54], I32, tag="gt", name="hi_i")
            lo_i = pt.tile([128, NCHUNK, 54], I32, tag="dpk", name="lo_i")
            pxhl_ = pf.tile([128, NCHUNK, 108], BF16, tag="pxhl")
            l_ = pt.tile([128, NCHUNK, 108], F32, tag="gt", name="l_")
            l1_ = pt.tile([128, NCHUNK, 108], F32, tag="hw", name="l1_")
            am_ = pt.tile([128, NCHUNK, 54], F32, tag="dpk", name="am_")
            bm_ = pt.tile([128, NCHUNK, 54], F32, tag="bm")
            wf_ = pf.tile([128, NCHUNK, 54, 4], BF16, tag="wf")
            wrd_all = pf.tile([128, NCHUNK, 432], I16, tag="wrd")

            def load_seg(c0, c1):
                s0, s1 = c0 * 128, c1 * 128
                nch = c1 - c0
                nc.sync.dma_start(
                    offs_t[:, c0:c1],
                    offs[s0:s1].rearrange("(c p) f -> p c f", p=128))
                nc.scalar.dma_start(
                    bases_t[:, c0:c1],
                    bases[s0:s1].rearrange("(c p) f -> p c f", p=128))
                nc.scalar.dma_start(
                    msk_t[:, c0:c1],
                    msk[s0:s1].rearrange("(c p) f -> p c f", p=128))
                nc.sync.dma_start(
                    dpk_t[:, c0:c1],
                    dpk[s0:s1].rearrange("(c p) f -> p c f", p=128))

            def field_seg(c0, c1):
                c = slice(c0, c1)
                nc.vector.tensor_tensor(out=hw_[:, c], in0=offs_t[:, c],
                                        in1=bases_t[:, c], op=AT.add)
                nc.vector.tensor_scalar(out=hw_[:, c], in0=hw_[:, c], scalar1=49.0,
                                        scalar2=0.0, op0=AT.min, op1=AT.max)
                nc.vector.tensor_copy(out=ti_[:, c], in_=hw_[:, c])
                nc.scalar.activation(out=tf_[:, c], in_=ti_[:, c], func=AF.Copy)
                nc.vector.tensor_tensor(out=gt_[:, c], in0=tf_[:, c],
                                        in1=hw_[:, c], op=AT.is_gt)
                nc.vector.tensor_tensor(out=tf_[:, c], in0=tf_[:, c],
                                        in1=gt_[:, c], op=AT.subtract)

                # px = floor_h * 52 + floor_w + dpk  (exact small ints in f32)
                nc.vector.tensor_scalar(out=px_[:, c], in0=tf_[:, c, :54],
                                        scalar1=52.0, scalar2=None, op0=AT.mult)
                nc.vector.tensor_tensor(out=px_[:, c], in0=px_[:, c],
                                        in1=tf_[:, c, 54:], op=AT.add)
                nc.vector.tensor_tensor(out=px_[:, c], in0=px_[:, c],
                                        in1=dpk_t[:, c], op=AT.add)

                # split px = hi*128 + lo so the wrap matmuls can run in bf16
                # (hi <= 211 and lo < 128 are bf16-exact).  px is an exact
                # integer in f32, so the i32 conversion is exact and hi/lo
                # are just a shift and a mask.
                nc.vector.tensor_copy(out=ph_i[:, c], in_=px_[:, c])
                nc.vector.tensor_scalar(out=hi_i[:, c], in0=ph_i[:, c], scalar1=7,
                                        scalar2=None, op0=AT.arith_shift_right)
                nc.vector.tensor_scalar(out=lo_i[:, c], in0=ph_i[:, c], scalar1=127,
                                        scalar2=None, op0=AT.bitwise_and)
                nc.vector.tensor_copy(out=pxhl_[:, c, :54], in_=hi_i[:, c])
                nc.vector.tensor_copy(out=pxhl_[:, c, 54:], in_=lo_i[:, c])

                nc.vector.tensor_tensor(out=l_[:, c], in0=hw_[:, c],
                                        in1=tf_[:, c], op=AT.subtract)
                nc.scalar.activation(out=l1_[:, c], in_=l_[:, c], func=AF.Copy,
                                     scale=-1.0, bias=1.0)

                # corner weights, col-major: wf[p, ci, (pl,k), t] bf16
                nc.vector.tensor_tensor(out=am_[:, c], in0=l1_[:, c, :54],
                                        in1=msk_t[:, c], op=AT.mult)
                nc.vector.tensor_tensor(out=bm_[:, c], in0=l_[:, c, :54],
                                        in1=msk_t[:, c], op=AT.mult)
                for t, (ab, lw0) in enumerate([(am_, l1_), (am_, l_),
                                               (bm_, l1_), (bm_, l_)]):
                    nc.vector.tensor_tensor(out=wf_[:, c, :, t], in0=ab[:, c],
                                            in1=lw0[:, c, 54:], op=AT.mult)

            # ---- wrap: one chunk's px into the dma_gather int16 index
            # layout, materialized directly in all 5 idx bands (parts 0:16 +
            # queue bands 16:32, 48:64, 80:96, 112:128): the banded
            # selection matmuls write wrp[band0+r, q*54+col] = px[q*16+r,
            # col]; one 128-partition DVE copy converts to i16 in the
            # wrapped (col*8+q) order.
            def wrap_chunk(ci):
                wrp = psW.tile([128, 432], F32, tag="wrap", space="PSUM",
                               name=f"wrp_{ci}")
                for q in range(8):
                    # PSUM-accumulated recombine: 128*hi (scaled selection
                    # matrix, cols q*128..) + lo (plain selection, cols
                    # (8+q)*128..)
                    nc.tensor.matmul(out=wrp[:, q * 54:(q + 1) * 54],
                                     lhsT=idf_t[:, q * 128:(q + 1) * 128],
                                     rhs=pxhl_[:, ci, :54], start=True, stop=False)
                    nc.tensor.matmul(out=wrp[:, q * 54:(q + 1) * 54],
                                     lhsT=idf_t[:, (8 + q) * 128:(9 + q) * 128],
                                     rhs=pxhl_[:, ci, 54:], start=False, stop=True)
                nc.vector.tensor_copy(
                    out=wrd_all[:, ci].rearrange("p (col q) -> p q col", q=8),
                    in_=wrp[:].rearrange("p (q col) -> p q col", col=54))

            # segment A: chunks 0-1 ready ASAP; wrap them; the rest of the
            # field phase is emitted just-in-time inside the chunk loop so
            # the scheduler can't starve the early critical chain with it.
            SEGA = 2
            load_seg(0, SEGA)
            field_seg(0, SEGA)
            for ci in range(SEGA):
                wrap_chunk(ci)
            load_seg(SEGA, NCHUNK)
            field_seg(SEGA, NCHUNK)
            if debug:
                nc.sync.dma_start(dbg_px[:], px_[:, 0, :])
                nc.sync.dma_start(dbg_wr[:], wrd_all[16:32, 0, :])
                nc.sync.dma_start(dbg_wf[:], wf_[:, 0, :, :])

            vs_cur = {}
            NG = 4                                     # max chunks per conv group
            for ci in range(NCHUNK):
                gi0 = ci % 4 == 0                      # group leader chunk
                ng = 4 if ci < 16 else 2               # group size
                c0 = ci - (ci % 4 if ci < 16 else ci - 16)

                # wrap lookahead: keep the PE two chunks ahead of the gathers
                if SEGA <= ci + SEGA < NCHUNK:
                    wrap_chunk(ci + SEGA)

                # ---- gather: one 256B bf16 quad row per (plane, tap, sample)
                G = pg.tile([128, NCOL, ROW], GATHER_DT, tag="G")
                col0 = 0
                if "gather" in skip:
                    nc.vector.memset(G[:, :1, :1], 0)
                for ncols, qn in (CALL_SCHED if "gather" not in skip else []):
                    nidx = ncols * 128
                    gi = nc.gpsimd.dma_gather(
                        G[:, col0:col0 + ncols, :],
                        xq[:],
                        wrd_all[:, ci, col0 * 8: col0 * 8 + nidx // 16],
                        nidx, nidx, ROW, queue_num=qn,
                        single_packet=False)
                    add_dep_helper(gi.ins, lib_inst.ins, sync=False,
                                   reason="mlp library before dma_gather")
                    col0 += ncols

                # ---- corner-weight multiply + corner sum (DVE)
                if gi0:
                    vs_cur[0] = pvs.tile([128, 2, NG, 896], BF16, tag="vs",
                                         name=f"vs_{ci}")
                    nc.vector.memset(vs_cur[0][:, :, :, 864:], 0)
                if "vmul" not in skip:
                    # expand corner weights over c on the (idle) ACT engine so
                    # the DVE multiply gets two contiguous operands; per plane
                    # to halve the buffer
                    V4g = pv.tile([128, NCOL, 4, 32], BF16, tag="v4g", bufs=1)
                    for pl in range(2):
                        sl = slice(pl * K, (pl + 1) * K)
                        wfx = pv.tile([128, K, 4, 32], BF16, tag="wfx",
                                      name=f"wfx_{ci}_{pl}")
                        nc.scalar.activation(
                            out=wfx[:], in_=wf_[:, ci, sl].to_broadcast([128, K, 4, 32]),
                            func=AF.Copy)
                        nc.vector.tensor_tensor(
                            out=V4g[:, sl],
                            in0=G[:, sl].rearrange("p col (t c) -> p col t c", c=32),
                            in1=wfx[:],
                            op=AT.mult)
                    # pairwise corner sums: (t0+t1) + (t2+t3)
                    with nc.allow_low_precision("4-term bf16 corner sum"):
                        t01 = pv.tile([128, NCOL, 32], BF16, tag="t01", bufs=1)
                        nc.vector.tensor_tensor(out=t01[:], in0=V4g[:, :, 0, :],
                                                in1=V4g[:, :, 1, :], op=AT.add)
                        t23 = pv.tile([128, NCOL, 32], BF16, tag="t23", bufs=1)
                        nc.vector.tensor_tensor(out=t23[:], in0=V4g[:, :, 2, :],
                                                in1=V4g[:, :, 3, :], op=AT.add)
                        for pl in range(2):
                            sl = slice(pl * K, (pl + 1) * K)
                            nc.vector.tensor_tensor(
                                out=vs_cur[0][:, pl, ci - c0, :864],
                                in0=t01[:, sl], in1=t23[:, sl], op=AT.add)

                # ---- group end: XBAR transpose + conv matmuls
                if ci - c0 == ng - 1 and "conv" not in skip:
                    r0g = c0 * 128
                    Vs = vs_cur[0]
                    VtT = ptt.tile([128, 2 * NG * 7, 128], BF16, tag="vt",
                                   name=f"vt_{ci}")
                    nc.sync.dma_start_transpose(
                        VtT[:], Vs[:].rearrange("p a b c -> p (a b c)"))
                    rhs4 = VtT[:].rearrange("p (pl c4 g) s -> p pl g c4 s", pl=2, g=7)
                    for pl in range(2):
                        cp = psC.tile([64, NG * 128], F32, tag="conv", space="PSUM",
                                      name=f"cp_{ci}_{pl}")
                        for g in range(7):
                            nc.tensor.matmul(out=cp[:, :ng * 128],
                                             lhsT=wt_t[:, g * 64:(g + 1) * 64],
                                             rhs=rhs4[:, pl, g, :ng],
                                             start=(g == 0), stop=(g == 6))
                        ou = po.tile([64, NG * 128], F32, tag="ou",
                                     name=f"ou_{ci}_{pl}")
                        nc.vector.tensor_scalar(out=ou[:, :ng * 128], in0=cp[:, :ng * 128],
                                                scalar1=bia_t[:64, :],
                                                scalar2=None, op0=AT.add)
                        nc.scalar.dma_start(out[pl, :, r0g:r0g + ng * 128], ou[:, :ng * 128])

    nc.compile()
    return nc


def _prep_static():
    """Input-independent constant tensors."""
    yy, xx = np.meshgrid(np.arange(H), np.arange(W), indexing="ij")
    yy = yy.reshape(-1).astype(np.float32)
    xx = xx.reshape(-1).astype(np.float32)
    kd = (np.arange(K) // 9).astype(np.float32)
    kh = ((np.arange(K) // 3) % 3).astype(np.float32)
    kw = (np.arange(K) % 3).astype(np.float32)

    bases = np.zeros((S, 108), np.float32)
    for pl in range(2):
        bases[:, pl * K:(pl + 1) * K] = yy[:, None] + kh[None, :]
        bases[:, 54 + pl * K:54 + (pl + 1) * K] = xx[:, None] + kw[None, :]

    # banded wrap selection: idf[s, q*128 + band0 + r] = w for s = q*16 + r,
    # bands at partitions {0, 16, 48, 80, 112} (CoreSim + 4 SWDGE queues).
    # Blocks 0..7 carry weight 128 (px hi part), blocks 8..15 weight 1 (lo).
    idf = np.zeros((128, 16, 128), np.float32)
    for q in range(8):
        for r in range(16):
            for band0 in (0, 16, 48, 80, 112):
                idf[q * 16 + r, q, band0 + r] = 128.0
                idf[q * 16 + r, 8 + q, band0 + r] = 1.0
    idf = idf.reshape(128, 16 * 128).astype(ml_dtypes.bfloat16)
    return bases, kd, idf


def _prep_weights(weight, bias):
    # wt rows kc = k*32 + c ; wt[kc, o] = weight[o, c, k]
    wk = weight.reshape(COUT, CIN, K)          # [o, c, k]
    wt = np.zeros((896, COUT), np.float32)
    wt[:864] = wk.transpose(2, 1, 0).reshape(864, COUT)   # [k, c, o] -> rows k*32+c
    # pack [7, 128, 64] -> [128, 7*64] for a single contiguous DMA
    wt = wt.reshape(7, 128, COUT).transpose(1, 0, 2).reshape(128, 7 * COUT)
    wt = np.ascontiguousarray(wt).astype(ml_dtypes.bfloat16)
    bia = bias.reshape(64, 1).astype(np.float32)
    return wt, bia


def _prep_quad(x):
    """x [B, C, D, H, W] -> quad [B, PLANE_PX, 128] bfloat16."""
    xp = np.zeros((B, DP, HPAD + 1, WPAD + 1, CIN), np.float32)
    xp[:, 1:1 + D, 1:1 + H, 1:1 + W, :] = x.transpose(0, 2, 3, 4, 1)
    q = np.empty((B, DP, HPAD, WPAD, 4, CIN), np.float32)
    for t, (cy, j) in enumerate([(0, 0), (0, 1), (1, 0), (1, 1)]):
        q[..., t, :] = xp[:, :, cy:cy + HPAD, j:j + WPAD, :]
    q = q.reshape(B, PLANE_PX, ROW)
    if GATHER_DT == BF16:
        q = q.astype(ml_dtypes.bfloat16)
    return q


def make_in_maps(input, offset, mask, weight, bias):
    if "static" not in _CACHE:
        _CACHE["static"] = _prep_static()
    bases, kd, idf = _CACHE["static"]
    wt, bia = _prep_weights(weight, bias)
    quad = _prep_quad(input)

    offr = offset.reshape(B, K, 2, D, S)   # [b, k, comp, z, s]
    mr = mask.reshape(B, K, D, S)

    in_maps = []
    for core in range(N_CORES):
        bidx = core // 4
        z0 = (2 * core) % 8
        offs_c = np.empty((S, 108), np.float32)
        msk_c = np.empty((S, 54), np.float32)
        dpk_c = np.empty((S, 54), np.float32)
        for pl, z in enumerate((z0, z0 + 1)):
            offs_c[:, pl * K:(pl + 1) * K] = offr[bidx, :, 0, z, :].T
            offs_c[:, 54 + pl * K:54 + (pl + 1) * K] = offr[bidx, :, 1, z, :].T
            msk_c[:, pl * K:(pl + 1) * K] = mr[bidx, :, z, :].T
            dpk_c[:, pl * K:(pl + 1) * K] = ((z + kd) * (HPAD * WPAD))[None, :]
        in_maps.append({
            "xq": quad[bidx],
            "offs": offs_c,
            "msk": msk_c,
            "bases": bases,
            "dpk": dpk_c,
            "wt": wt,
            "bia": bia,
            "idf": idf,
        })
    return in_maps


def kernel(input, offset, mask, weight, bias):
    input = np.ascontiguousarray(input, np.float32)
    offset = np.ascontiguousarray(offset, np.float32)
    mask = np.ascontiguousarray(mask, np.float32)
    weight = np.ascontiguousarray(weight, np.float32)
    bias = np.ascontiguousarray(bias, np.float32)

    if "nc" not in _CACHE:
        _CACHE["nc"] = build_nc()
    nc = _CACHE["nc"]
    in_maps = make_in_maps(input, offset, mask, weight, bias)

    res = run_bass_kernel_spmd(nc, in_maps, core_ids=list(range(N_CORES)))

    out = np.empty((B, COUT, D, H, W), np.float32)
    for core in range(N_CORES):
        bidx = core // 4
        z0 = (2 * core) % 8
        o = np.asarray(res.results[core]["out"], np.float32)   # [2, 64, S]
        out[bidx, :, z0] = o[0].reshape(COUT, H, W)
        out[bidx, :, z0 + 1] = o[1].reshape(COUT, H, W)
    return out



# revision 23
# speedup vs baseline: 1.0648x; 1.0231x over previous
"""Deformable 3D convolution (DeformConv3d) on 8 TRN2 NeuronCores via Bass/Tile.

Strategy (data-parallel over the 16 (b, z) output planes, 2 per core):
  - Host packs x into a zero-padded bf16 "quad image": for every padded pixel
    (dp, hp, wp) a 128-element row [t=(cy,j) major, c minor] holding the
    2x2 bilinear corner patch across all 32 channels.  One dma_gather
    descriptor (256B) fetches all 4 corners x 32 channels for one
    (tap, sample) pair.
  - Device, per core: field phase computes floor/frac/corner weights for all
    18 chunks in a handful of large DVE ops; an upfront wrap phase turns px
    (split hi*128+lo so the selection matmuls run in bf16, recombined by PSUM
    accumulation) into the int16 gather-index layout, materialized directly
    in all SWDGE queue idx bands by banded selection matmuls; per 128-sample
    chunk dma_gather (8 calls round-robin over the 4 queues, queue loads
    alternated per chunk for balance) lands G[s, (pl,k), (t,c)] bf16; the
    corner weights are c-expanded on the ACT engine so the DVE multiply gets
    two contiguous bf16 operands, pairwise adds sum the 4 corners; one XBAR
    DMA-transpose per 4-chunk group flips both planes' weighted sums into
    [kc, s] layout (no PE data transposes); the conv is 7 accumulating bf16
    matmuls of 512 columns per (plane, group), then bias-add and store.

  Bottleneck note: steady state is paced by SWDGE descriptor generation
    (~8.5 ns/descriptor per queue Q7 core, ~0.76 us/call, 4 queues, 1024
    idx/call hardware cap) at ~15-16 us per 6912-descriptor chunk; compute,
    DMA transfer and the conv are all overlapped underneath it.
"""

import numpy as np
import ml_dtypes

import concourse.bass as bass
import concourse.bacc as bacc
import concourse.mybir as mybir
from concourse import tile
from concourse import library_config
from concourse.bass_utils import run_bass_kernel_spmd
from concourse.tile_rust import add_dep_helper

F32 = mybir.dt.float32
BF16 = mybir.dt.bfloat16
I32 = mybir.dt.int32
I16 = mybir.dt.int16
AT = mybir.AluOpType
AF = mybir.ActivationFunctionType
AX = mybir.AxisListType

# problem constants
B, CIN, D, H, W = 2, 32, 8, 48, 48
K, COUT = 27, 64
S = H * W                      # 2304 samples per plane
DP, HPAD, WPAD = 10, 52, 52    # padded depth/rows/cols
PLANE_PX = DP * HPAD * WPAD    # 27040 quad rows per batch
ROW = 128                      # quad row payload elems (4 corners x 32 ch)
NCHUNK = S // 128              # 18
NCOL = 2 * K                   # 54 = (plane, tap) columns per chunk
# dma_gather call splits (<=1024 idx each).  Queue 0's descriptor
# generation runs synchronously ON the Pool engine (~10.6 ns/idx of engine
# residency, observed on HW); queues 1-3 hand off asynchronously (~600 ns
# dispatch) and generate in the background.  So: queues 1-3 carry most of
# the load (dispatched first), queue 0 a small tail share (dispatched
# last, so its engine-blocking overlaps the async queues' background
# generation).
CALL_SCHED = [(7, 1), (7, 2), (7, 3), (6, 0), (7, 1), (7, 2), (7, 3), (6, 0)]
CALL_COLS = [c for c, _ in CALL_SCHED]
N_CORES = 8
NQ = 4

_CACHE = {}
GATHER_DT = BF16               # quad image + G dtype


def build_nc(skip=(), debug=False):
    nc = bacc.Bacc("TRN2", target_bir_lowering=False, debug=False,
                   num_swdge_queues=NQ)
    xq = nc.dram_tensor("xq", [PLANE_PX, ROW], GATHER_DT, kind="ExternalInput")
    offs = nc.dram_tensor("offs", [S, 108], F32, kind="ExternalInput")
    msk = nc.dram_tensor("msk", [S, 54], F32, kind="ExternalInput")
    bases = nc.dram_tensor("bases", [S, 108], F32, kind="ExternalInput")
    dpk = nc.dram_tensor("dpk", [S, 54], F32, kind="ExternalInput")
    wt = nc.dram_tensor("wt", [128, 7 * 64], BF16, kind="ExternalInput")
    bia = nc.dram_tensor("bia", [64, 1], F32, kind="ExternalInput")
    idf = nc.dram_tensor("idf", [128, 16 * 128], BF16, kind="ExternalInput")
    out = nc.dram_tensor("out", [2, 64, S], F32, kind="ExternalOutput")
    if debug:
        dbg_px = nc.dram_tensor("dbg_px", [128, 54], F32, kind="ExternalOutput")
        dbg_wr = nc.dram_tensor("dbg_wr", [16, 432], I16, kind="ExternalOutput")
        dbg_wf = nc.dram_tensor("dbg_wf", [128, 4, 54], BF16, kind="ExternalOutput")

    with tile.TileContext(nc) as tc:
        with (
            tc.tile_pool(name="const", bufs=1) as pc,
            tc.tile_pool(name="fldT", bufs=1) as pt,   # transient field tensors
            tc.tile_pool(name="fldP", bufs=1) as pf,   # persistent px / wf
            tc.tile_pool(name="gg", bufs=3) as pg,
            tc.tile_pool(name="v4", bufs=2) as pv,
            tc.tile_pool(name="vs", bufs=2) as pvs,
            tc.tile_pool(name="vt", bufs=2) as ptt,
            tc.tile_pool(name="oo", bufs=2) as po,
            tc.tile_pool(name="psW", bufs=6, space="PSUM") as psW,
            tc.tile_pool(name="psC", bufs=2, space="PSUM") as psC,
        ):
            wt_t = pc.tile([128, 7 * 64], BF16)
            nc.sync.dma_start(wt_t[:], wt[:])
            bia_t = pc.tile([64, 1], F32)
            nc.sync.dma_start(bia_t[:], bia[:])
            idf_t = pc.tile([128, 16 * 128], BF16)
            nc.sync.dma_start(idf_t[:], idf[:])
            lib_inst = nc.gpsimd.load_library(library_config.mlp)

            # ---- input loads + field phase, in two segments so chunk 0's
            # gathers can dispatch long before the full field phase ends.
            offs_t = pt.tile([128, NCHUNK, 108], F32, tag="offs")
            bases_t = pt.tile([128, NCHUNK, 108], F32, tag="bases")
            msk_t = pt.tile([128, NCHUNK, 54], F32, tag="msk")
            dpk_t = pt.tile([128, NCHUNK, 54], F32, tag="dpk")
            hw_ = pt.tile([128, NCHUNK, 108], F32, tag="hw")
            ti_ = pt.tile([128, NCHUNK, 108], I32, tag="offs", name="ti_")
            tf_ = pt.tile([128, NCHUNK, 108], F32, tag="bases", name="tf_")
            gt_ = pt.tile([128, NCHUNK, 108], F32, tag="gt")
            px_ = pt.tile([128, NCHUNK, 54], F32, tag="bm", name="px_")
            ph_i = pt.tile([128, NCHUNK, 54], I32, tag="offs", name="ph_i")
            hi_i = pt.tile([128, NCHUNK, 54], I32, tag="gt", name="hi_i")
            lo_i = pt.tile([128, NCHUNK, 54], I32, tag="dpk", name="lo_i")
            pxhl_ = pf.tile([128, NCHUNK, 108], BF16, tag="pxhl")
            l_ = pt.tile([128, NCHUNK, 108], F32, tag="gt", name="l_")
            l1_ = pt.tile([128, NCHUNK, 108], F32, tag="hw", name="l1_")
            am_ = pt.tile([128, NCHUNK, 54], F32, tag="dpk", name="am_")
            bm_ = pt.tile([128, NCHUNK, 54], F32, tag="bm")
            wf_ = pf.tile([128, NCHUNK, 54, 4], BF16, tag="wf")
            wrd_all = pf.tile([128, NCHUNK, 432], I16, tag="wrd")

            def load_seg(c0, c1):
                s0, s1 = c0 * 128, c1 * 128
                nch = c1 - c0
                nc.sync.dma_start(
                    offs_t[:, c0:c1],
                    offs[s0:s1].rearrange("(c p) f -> p c f", p=128))
                nc.scalar.dma_start(
                    bases_t[:, c0:c1],
                    bases[s0:s1].rearrange("(c p) f -> p c f", p=128))
                nc.scalar.dma_start(
                    msk_t[:, c0:c1],
                    msk[s0:s1].rearrange("(c p) f -> p c f", p=128))
                nc.sync.dma_start(
                    dpk_t[:, c0:c1],
                    dpk[s0:s1].rearrange("(c p) f -> p c f", p=128))

            def field_seg(c0, c1):
                c = slice(c0, c1)
                nc.vector.tensor_tensor(out=hw_[:, c], in0=offs_t[:, c],
                                        in1=bases_t[:, c], op=AT.add)
                nc.vector.tensor_scalar(out=hw_[:, c], in0=hw_[:, c], scalar1=49.0,
                                        scalar2=0.0, op0=AT.min, op1=AT.max)
                nc.vector.tensor_copy(out=ti_[:, c], in_=hw_[:, c])
                nc.scalar.activation(out=tf_[:, c], in_=ti_[:, c], func=AF.Copy)
                nc.vector.tensor_tensor(out=gt_[:, c], in0=tf_[:, c],
                                        in1=hw_[:, c], op=AT.is_gt)
                nc.vector.tensor_tensor(out=tf_[:, c], in0=tf_[:, c],
                                        in1=gt_[:, c], op=AT.subtract)

                # px = floor_h * 52 + floor_w + dpk  (exact small ints in f32)
                nc.vector.tensor_scalar(out=px_[:, c], in0=tf_[:, c, :54],
                                        scalar1=52.0, scalar2=None, op0=AT.mult)
                nc.vector.tensor_tensor(out=px_[:, c], in0=px_[:, c],
                                        in1=tf_[:, c, 54:], op=AT.add)
                nc.vector.tensor_tensor(out=px_[:, c], in0=px_[:, c],
                                        in1=dpk_t[:, c], op=AT.add)

                # split px = hi*128 + lo so the wrap matmuls can run in bf16
                # (hi <= 211 and lo < 128 are bf16-exact).  px is an exact
                # integer in f32, so the i32 conversion is exact and hi/lo
                # are just a shift and a mask.
                nc.vector.tensor_copy(out=ph_i[:, c], in_=px_[:, c])
                nc.vector.tensor_scalar(out=hi_i[:, c], in0=ph_i[:, c], scalar1=7,
                                        scalar2=None, op0=AT.arith_shift_right)
                nc.vector.tensor_scalar(out=lo_i[:, c], in0=ph_i[:, c], scalar1=127,
                                        scalar2=None, op0=AT.bitwise_and)
                nc.vector.tensor_copy(out=pxhl_[:, c, :54], in_=hi_i[:, c])
                nc.vector.tensor_copy(out=pxhl_[:, c, 54:], in_=lo_i[:, c])

            def field_seg_wf(c0, c1):
                # corner-weight part: needed only by the vmul, ~a chunk
                # cadence after the px part gates the wrap + gather.
                c = slice(c0, c1)
                nc.vector.tensor_tensor(out=l_[:, c], in0=hw_[:, c],
                                        in1=tf_[:, c], op=AT.subtract)
                nc.scalar.activation(out=l1_[:, c], in_=l_[:, c], func=AF.Copy,
                                     scale=-1.0, bias=1.0)

                # corner weights, col-major: wf[p, ci, (pl,k), t] bf16
                nc.vector.tensor_tensor(out=am_[:, c], in0=l1_[:, c, :54],
                                        in1=msk_t[:, c], op=AT.mult)
                nc.vector.tensor_tensor(out=bm_[:, c], in0=l_[:, c, :54],
                                        in1=msk_t[:, c], op=AT.mult)
                for t, (ab, lw0) in enumerate([(am_, l1_), (am_, l_),
                                               (bm_, l1_), (bm_, l_)]):
                    nc.vector.tensor_tensor(out=wf_[:, c, :, t], in0=ab[:, c],
                                            in1=lw0[:, c, 54:], op=AT.mult)

            # ---- wrap: one chunk's px into the dma_gather int16 index
            # layout, materialized directly in all 5 idx bands (parts 0:16 +
            # queue bands 16:32, 48:64, 80:96, 112:128): the banded
            # selection matmuls write wrp[band0+r, q*54+col] = px[q*16+r,
            # col]; one 128-partition DVE copy converts to i16 in the
            # wrapped (col*8+q) order.
            def wrap_chunk(ci):
                wrp = psW.tile([128, 432], F32, tag="wrap", space="PSUM",
                               name=f"wrp_{ci}")
                for q in range(8):
                    # PSUM-accumulated recombine: 128*hi (scaled selection
                    # matrix, cols q*128..) + lo (plain selection, cols
                    # (8+q)*128..)
                    nc.tensor.matmul(out=wrp[:, q * 54:(q + 1) * 54],
                                     lhsT=idf_t[:, q * 128:(q + 1) * 128],
                                     rhs=pxhl_[:, ci, :54], start=True, stop=False)
                    nc.tensor.matmul(out=wrp[:, q * 54:(q + 1) * 54],
                                     lhsT=idf_t[:, (8 + q) * 128:(9 + q) * 128],
                                     rhs=pxhl_[:, ci, 54:], start=False, stop=True)
                nc.vector.tensor_copy(
                    out=wrd_all[:, ci].rearrange("p (col q) -> p q col", q=8),
                    in_=wrp[:].rearrange("p (q col) -> p q col", col=54))

            # segment A: chunks 0-1 ready ASAP; wrap them; the rest of the
            # field phase is emitted just-in-time inside the chunk loop so
            # the scheduler can't starve the early critical chain with it.
            SEGA = 2
            load_seg(0, SEGA)
            field_seg(0, SEGA)
            for ci in range(SEGA):
                wrap_chunk(ci)
            load_seg(SEGA, NCHUNK)
            field_seg(SEGA, NCHUNK)
            field_seg_wf(0, SEGA)
            field_seg_wf(SEGA, NCHUNK)
            if debug:
                nc.sync.dma_start(dbg_px[:], px_[:, 0, :])
                nc.sync.dma_start(dbg_wr[:], wrd_all[16:32, 0, :])
                nc.sync.dma_start(dbg_wf[:], wf_[:, 0, :, :])

            vs_cur = {}
            NG = 4                                     # max chunks per conv group
            for ci in range(NCHUNK):
                gi0 = ci % 4 == 0                      # group leader chunk
                ng = 4 if ci < 16 else 2               # group size
                c0 = ci - (ci % 4 if ci < 16 else ci - 16)

                # wrap lookahead: keep the PE two chunks ahead of the gathers
                if SEGA <= ci + SEGA < NCHUNK:
                    wrap_chunk(ci + SEGA)

                # ---- gather: one 256B bf16 quad row per (plane, tap, sample)
                G = pg.tile([128, NCOL, ROW], GATHER_DT, tag="G")
                col0 = 0
                if "gather" in skip:
                    nc.vector.memset(G[:, :1, :1], 0)
                for ncols, qn in (CALL_SCHED if "gather" not in skip else []):
                    nidx = ncols * 128
                    gi = nc.gpsimd.dma_gather(
                        G[:, col0:col0 + ncols, :],
                        xq[:],
                        wrd_all[:, ci, col0 * 8: col0 * 8 + nidx // 16],
                        nidx, nidx, ROW, queue_num=qn,
                        single_packet=False)
                    add_dep_helper(gi.ins, lib_inst.ins, sync=False,
                                   reason="mlp library before dma_gather")
                    col0 += ncols

                # ---- corner-weight multiply + corner sum (DVE)
                if gi0:
                    vs_cur[0] = pvs.tile([128, 2, NG, 896], BF16, tag="vs",
                                         name=f"vs_{ci}")
                    nc.vector.memset(vs_cur[0][:, :, :, 864:], 0)
                if "vmul" not in skip:
                    # expand corner weights over c on the (idle) ACT engine so
                    # the DVE multiply gets two contiguous operands; per plane
                    # to halve the buffer
                    V4g = pv.tile([128, NCOL, 4, 32], BF16, tag="v4g", bufs=1)
                    for pl in range(2):
                        sl = slice(pl * K, (pl + 1) * K)
                        wfx = pv.tile([128, K, 4, 32], BF16, tag="wfx",
                                      name=f"wfx_{ci}_{pl}")
                        nc.scalar.activation(
                            out=wfx[:], in_=wf_[:, ci, sl].to_broadcast([128, K, 4, 32]),
                            func=AF.Copy)
                        nc.vector.tensor_tensor(
                            out=V4g[:, sl],
                            in0=G[:, sl].rearrange("p col (t c) -> p col t c", c=32),
                            in1=wfx[:],
                            op=AT.mult)
                    # pairwise corner sums: (t0+t1) + (t2+t3)
                    with nc.allow_low_precision("4-term bf16 corner sum"):
                        t01 = pv.tile([128, NCOL, 32], BF16, tag="t01", bufs=1)
                        nc.vector.tensor_tensor(out=t01[:], in0=V4g[:, :, 0, :],
                                                in1=V4g[:, :, 1, :], op=AT.add)
                        t23 = pv.tile([128, NCOL, 32], BF16, tag="t23", bufs=1)
                        nc.vector.tensor_tensor(out=t23[:], in0=V4g[:, :, 2, :],
                                                in1=V4g[:, :, 3, :], op=AT.add)
                        for pl in range(2):
                            sl = slice(pl * K, (pl + 1) * K)
                            nc.vector.tensor_tensor(
                                out=vs_cur[0][:, pl, ci - c0, :864],
                                in0=t01[:, sl], in1=t23[:, sl], op=AT.add)

                # ---- group end: XBAR transpose + conv matmuls
                if ci - c0 == ng - 1 and "conv" not in skip:
                    r0g = c0 * 128
                    Vs = vs_cur[0]
                    VtT = ptt.tile([128, 2 * NG * 7, 128], BF16, tag="vt",
                                   name=f"vt_{ci}")
                    nc.sync.dma_start_transpose(
                        VtT[:], Vs[:].rearrange("p a b c -> p (a b c)"))
                    rhs4 = VtT[:].rearrange("p (pl c4 g) s -> p pl g c4 s", pl=2, g=7)
                    for pl in range(2):
                        cp = psC.tile([64, NG * 128], F32, tag="conv", space="PSUM",
                                      name=f"cp_{ci}_{pl}")
                        for g in range(7):
                            nc.tensor.matmul(out=cp[:, :ng * 128],
                                             lhsT=wt_t[:, g * 64:(g + 1) * 64],
                                             rhs=rhs4[:, pl, g, :ng],
                                             start=(g == 0), stop=(g == 6))
                        ou = po.tile([64, NG * 128], F32, tag="ou",
                                     name=f"ou_{ci}_{pl}")
                        nc.vector.tensor_scalar(out=ou[:, :ng * 128], in0=cp[:, :ng * 128],
                                                scalar1=bia_t[:64, :],
                                                scalar2=None, op0=AT.add)
                        nc.scalar.dma_start(out[pl, :, r0g:r0g + ng * 128], ou[:, :ng * 128])

    nc.compile()
    return nc


def _prep_static():
    """Input-independent constant tensors."""
    yy, xx = np.meshgrid(np.arange(H), np.arange(W), indexing="ij")
    yy = yy.reshape(-1).astype(np.float32)
    xx = xx.reshape(-1).astype(np.float32)
    kd = (np.arange(K) // 9).astype(np.float32)
    kh = ((np.arange(K) // 3) % 3).astype(np.float32)
    kw = (np.arange(K) % 3).astype(np.float32)

    bases = np.zeros((S, 108), np.float32)
    for pl in range(2):
        bases[:, pl * K:(pl + 1) * K] = yy[:, None] + kh[None, :]
        bases[:, 54 + pl * K:54 + (pl + 1) * K] = xx[:, None] + kw[None, :]

    # banded wrap selection: idf[s, q*128 + band0 + r] = w for s = q*16 + r,
    # bands at partitions {0, 16, 48, 80, 112} (CoreSim + 4 SWDGE queues).
    # Blocks 0..7 carry weight 128 (px hi part), blocks 8..15 weight 1 (lo).
    idf = np.zeros((128, 16, 128), np.float32)
    for q in range(8):
        for r in range(16):
            for band0 in (0, 16, 48, 80, 112):
                idf[q * 16 + r, q, band0 + r] = 128.0
                idf[q * 16 + r, 8 + q, band0 + r] = 1.0
    idf = idf.reshape(128, 16 * 128).astype(ml_dtypes.bfloat16)
    return bases, kd, idf


def _prep_weights(weight, bias):
    # wt rows kc = k*32 + c ; wt[kc, o] = weight[o, c, k]
    wk = weight.reshape(COUT, CIN, K)          # [o, c, k]
    wt = np.zeros((896, COUT), np.float32)
    wt[:864] = wk.transpose(2, 1, 0).reshape(864, COUT)   # [k, c, o] -> rows k*32+c
    # pack [7, 128, 64] -> [128, 7*64] for a single contiguous DMA
    wt = wt.reshape(7, 128, COUT).transpose(1, 0, 2).reshape(128, 7 * COUT)
    wt = np.ascontiguousarray(wt).astype(ml_dtypes.bfloat16)
    bia = bias.reshape(64, 1).astype(np.float32)
    return wt, bia


def _prep_quad(x):
    """x [B, C, D, H, W] -> quad [B, PLANE_PX, 128] bfloat16."""
    xp = np.zeros((B, DP, HPAD + 1, WPAD + 1, CIN), np.float32)
    xp[:, 1:1 + D, 1:1 + H, 1:1 + W, :] = x.transpose(0, 2, 3, 4, 1)
    q = np.empty((B, DP, HPAD, WPAD, 4, CIN), np.float32)
    for t, (cy, j) in enumerate([(0, 0), (0, 1), (1, 0), (1, 1)]):
        q[..., t, :] = xp[:, :, cy:cy + HPAD, j:j + WPAD, :]
    q = q.reshape(B, PLANE_PX, ROW)
    if GATHER_DT == BF16:
        q = q.astype(ml_dtypes.bfloat16)
    return q


def make_in_maps(input, offset, mask, weight, bias):
    if "static" not in _CACHE:
        _CACHE["static"] = _prep_static()
    bases, kd, idf = _CACHE["static"]
    wt, bia = _prep_weights(weight, bias)
    quad = _prep_quad(input)

    offr = offset.reshape(B, K, 2, D, S)   # [b, k, comp, z, s]
    mr = mask.reshape(B, K, D, S)

    in_maps = []
    for core in range(N_CORES):
        bidx = core // 4
        z0 = (2 * core) % 8
        offs_c = np.empty((S, 108), np.float32)
        msk_c = np.empty((S, 54), np.float32)
        dpk_c = np.empty((S, 54), np.float32)
        for pl, z in enumerate((z0, z0 + 1)):
            offs_c[:, pl * K:(pl + 1) * K] = offr[bidx, :, 0, z, :].T
            offs_c[:, 54 + pl * K:54 + (pl + 1) * K] = offr[bidx, :, 1, z, :].T
            msk_c[:, pl * K:(pl + 1) * K] = mr[bidx, :, z, :].T
            dpk_c[:, pl * K:(pl + 1) * K] = ((z + kd) * (HPAD * WPAD))[None, :]
        in_maps.append({
            "xq": quad[bidx],
            "offs": offs_c,
            "msk": msk_c,
            "bases": bases,
            "dpk": dpk_c,
            "wt": wt,
            "bia": bia,
            "idf": idf,
        })
    return in_maps


def kernel(input, offset, mask, weight, bias):
    input = np.ascontiguousarray(input, np.float32)
    offset = np.ascontiguousarray(offset, np.float32)
    mask = np.ascontiguousarray(mask, np.float32)
    weight = np.ascontiguousarray(weight, np.float32)
    bias = np.ascontiguousarray(bias, np.float32)

    if "nc" not in _CACHE:
        _CACHE["nc"] = build_nc()
    nc = _CACHE["nc"]
    in_maps = make_in_maps(input, offset, mask, weight, bias)

    res = run_bass_kernel_spmd(nc, in_maps, core_ids=list(range(N_CORES)))

    out = np.empty((B, COUT, D, H, W), np.float32)
    for core in range(N_CORES):
        bidx = core // 4
        z0 = (2 * core) % 8
        o = np.asarray(res.results[core]["out"], np.float32)   # [2, 64, S]
        out[bidx, :, z0] = o[0].reshape(COUT, H, W)
        out[bidx, :, z0 + 1] = o[1].reshape(COUT, H, W)
    return out



# revision 25
# speedup vs baseline: 1.0999x; 1.0329x over previous
"""Deformable 3D convolution (DeformConv3d) on 8 TRN2 NeuronCores via Bass/Tile.

Strategy (data-parallel over the 16 (b, z) output planes, 2 per core):
  - Host packs x into a zero-padded bf16 "quad image": for every padded pixel
    (dp, hp, wp) a 128-element row [t=(cy,j) major, c minor] holding the
    2x2 bilinear corner patch across all 32 channels.  One dma_gather
    descriptor (256B) fetches all 4 corners x 32 channels for one
    (tap, sample) pair.
  - Device, per core: field phase computes floor/frac/corner weights for all
    18 chunks in a handful of large DVE ops; an upfront wrap phase turns px
    (split hi*128+lo so the selection matmuls run in bf16, recombined by PSUM
    accumulation) into the int16 gather-index layout, materialized directly
    in all SWDGE queue idx bands by banded selection matmuls; per 128-sample
    chunk dma_gather (8 calls round-robin over the 4 queues, queue loads
    alternated per chunk for balance) lands G[s, (pl,k), (t,c)] bf16; the
    corner weights are c-expanded on the ACT engine so the DVE multiply gets
    two contiguous bf16 operands, pairwise adds sum the 4 corners; one XBAR
    DMA-transpose per 4-chunk group flips both planes' weighted sums into
    [kc, s] layout (no PE data transposes); the conv is 7 accumulating bf16
    matmuls of 512 columns per (plane, group), then bias-add and store.

  Bottleneck note: steady state is paced by SWDGE descriptor generation
    (~8.5 ns/descriptor per queue Q7 core, ~0.76 us/call, 4 queues, 1024
    idx/call hardware cap) at ~15-16 us per 6912-descriptor chunk; compute,
    DMA transfer and the conv are all overlapped underneath it.
"""

import numpy as np
import ml_dtypes

import concourse.bass as bass
import concourse.bacc as bacc
import concourse.mybir as mybir
from concourse import tile
from concourse import library_config
from concourse.bass_utils import run_bass_kernel_spmd
from concourse.tile_rust import add_dep_helper

F32 = mybir.dt.float32
BF16 = mybir.dt.bfloat16
I32 = mybir.dt.int32
I16 = mybir.dt.int16
AT = mybir.AluOpType
AF = mybir.ActivationFunctionType
AX = mybir.AxisListType

# problem constants
B, CIN, D, H, W = 2, 32, 8, 48, 48
K, COUT = 27, 64
S = H * W                      # 2304 samples per plane
DP, HPAD, WPAD = 10, 52, 52    # padded depth/rows/cols
PLANE_PX = DP * HPAD * WPAD    # 27040 quad rows per batch
ROW = 128                      # quad row payload elems (4 corners x 32 ch)
NCHUNK = S // 128              # 18
NCOL = 2 * K                   # 54 = (plane, tap) columns per chunk
# dma_gather call splits (<=1024 idx each).  Queue 0's descriptor
# generation runs synchronously ON the Pool engine (~10.6 ns/idx of engine
# residency, observed on HW); queues 1-3 hand off asynchronously (~600 ns
# dispatch) and generate in the background.  So: queues 1-3 carry most of
# the load (dispatched first), queue 0 a small tail share (dispatched
# last, so its engine-blocking overlaps the async queues' background
# generation).
CALL_SCHED = [(7, 1), (7, 2), (7, 3), (6, 0), (7, 1), (7, 2), (7, 3), (6, 0)]
CALL_COLS = [c for c, _ in CALL_SCHED]
N_CORES = 8
NQ = 4

_CACHE = {}
GATHER_DT = BF16               # quad image + G dtype


def build_nc(skip=(), debug=False):
    nc = bacc.Bacc("TRN2", target_bir_lowering=False, debug=False,
                   num_swdge_queues=NQ)
    xq = nc.dram_tensor("xq", [PLANE_PX, ROW], GATHER_DT, kind="ExternalInput")
    offs = nc.dram_tensor("offs", [S, 108], F32, kind="ExternalInput")
    msk = nc.dram_tensor("msk", [S, 54], F32, kind="ExternalInput")
    bases = nc.dram_tensor("bases", [S, 108], F32, kind="ExternalInput")
    dpk = nc.dram_tensor("dpk", [S, 54], F32, kind="ExternalInput")
    wt = nc.dram_tensor("wt", [128, 7 * 64], BF16, kind="ExternalInput")
    bia = nc.dram_tensor("bia", [64, 1], F32, kind="ExternalInput")
    idf = nc.dram_tensor("idf", [128, 16 * 128], BF16, kind="ExternalInput")
    out = nc.dram_tensor("out", [2, 64, S], F32, kind="ExternalOutput")
    if debug:
        dbg_px = nc.dram_tensor("dbg_px", [128, 54], F32, kind="ExternalOutput")
        dbg_wr = nc.dram_tensor("dbg_wr", [16, 432], I16, kind="ExternalOutput")
        dbg_wf = nc.dram_tensor("dbg_wf", [128, 4, 54], BF16, kind="ExternalOutput")

    with tile.TileContext(nc) as tc:
        with (
            tc.tile_pool(name="const", bufs=1) as pc,
            tc.tile_pool(name="fldT", bufs=1) as pt,   # transient field tensors
            tc.tile_pool(name="fldP", bufs=1) as pf,   # persistent px / wf
            tc.tile_pool(name="gg", bufs=3) as pg,
            tc.tile_pool(name="v4", bufs=2) as pv,
            tc.tile_pool(name="vs", bufs=2) as pvs,
            tc.tile_pool(name="vt", bufs=2) as ptt,
            tc.tile_pool(name="oo", bufs=2) as po,
            tc.tile_pool(name="psW", bufs=6, space="PSUM") as psW,
            tc.tile_pool(name="psC", bufs=2, space="PSUM") as psC,
        ):
            wt_t = pc.tile([128, 7 * 64], BF16)
            nc.sync.dma_start(wt_t[:], wt[:])
            bia_t = pc.tile([64, 1], F32)
            nc.sync.dma_start(bia_t[:], bia[:])
            idf_t = pc.tile([128, 16 * 128], BF16)
            nc.sync.dma_start(idf_t[:], idf[:])
            lib_inst = nc.gpsimd.load_library(library_config.mlp)

            # ---- input loads + field phase, in two segments so chunk 0's
            # gathers can dispatch long before the full field phase ends.
            offs_t = pt.tile([128, NCHUNK, 108], F32, tag="offs")
            bases_t = pt.tile([128, NCHUNK, 108], F32, tag="bases")
            msk_t = pt.tile([128, NCHUNK, 54], F32, tag="msk")
            dpk_t = pt.tile([128, NCHUNK, 54], F32, tag="dpk")
            hw_ = pt.tile([128, NCHUNK, 108], F32, tag="hw")
            ti_ = pt.tile([128, NCHUNK, 108], I32, tag="offs", name="ti_")
            tf_ = pt.tile([128, NCHUNK, 108], F32, tag="bases", name="tf_")
            gt_ = pt.tile([128, NCHUNK, 108], F32, tag="gt")
            px_ = pt.tile([128, NCHUNK, 54], F32, tag="bm", name="px_")
            ph_i = pt.tile([128, NCHUNK, 54], I32, tag="offs", name="ph_i")
            hi_i = pt.tile([128, NCHUNK, 54], I32, tag="gt", name="hi_i")
            lo_i = pt.tile([128, NCHUNK, 54], I32, tag="dpk", name="lo_i")
            pxhl_ = pf.tile([128, NCHUNK, 108], BF16, tag="pxhl")
            l_ = pt.tile([128, NCHUNK, 108], F32, tag="gt", name="l_")
            l1_ = pt.tile([128, NCHUNK, 108], F32, tag="hw", name="l1_")
            am_ = pt.tile([128, NCHUNK, 54], F32, tag="dpk", name="am_")
            bm_ = pt.tile([128, NCHUNK, 54], F32, tag="bm")
            wf_ = pf.tile([128, NCHUNK, 54, 4], BF16, tag="wf")
            wrd_all = pf.tile([128, NCHUNK, 432], I16, tag="wrd")

            def load_seg(c0, c1):
                s0, s1 = c0 * 128, c1 * 128
                nch = c1 - c0
                nc.sync.dma_start(
                    offs_t[:, c0:c1],
                    offs[s0:s1].rearrange("(c p) f -> p c f", p=128))
                nc.scalar.dma_start(
                    bases_t[:, c0:c1],
                    bases[s0:s1].rearrange("(c p) f -> p c f", p=128))
                nc.scalar.dma_start(
                    msk_t[:, c0:c1],
                    msk[s0:s1].rearrange("(c p) f -> p c f", p=128))
                nc.sync.dma_start(
                    dpk_t[:, c0:c1],
                    dpk[s0:s1].rearrange("(c p) f -> p c f", p=128))

            def field_seg(c0, c1):
                c = slice(c0, c1)
                nc.vector.tensor_tensor(out=hw_[:, c], in0=offs_t[:, c],
                                        in1=bases_t[:, c], op=AT.add)
                nc.vector.tensor_scalar(out=hw_[:, c], in0=hw_[:, c], scalar1=49.0,
                                        scalar2=0.0, op0=AT.min, op1=AT.max)
                nc.vector.tensor_copy(out=ti_[:, c], in_=hw_[:, c])
                nc.scalar.activation(out=tf_[:, c], in_=ti_[:, c], func=AF.Copy)
                nc.vector.tensor_tensor(out=gt_[:, c], in0=tf_[:, c],
                                        in1=hw_[:, c], op=AT.is_gt)
                nc.vector.tensor_tensor(out=tf_[:, c], in0=tf_[:, c],
                                        in1=gt_[:, c], op=AT.subtract)

                # px = floor_h * 52 + floor_w + dpk  (exact small ints in f32)
                nc.vector.tensor_scalar(out=px_[:, c], in0=tf_[:, c, :54],
                                        scalar1=52.0, scalar2=None, op0=AT.mult)
                nc.vector.tensor_tensor(out=px_[:, c], in0=px_[:, c],
                                        in1=tf_[:, c, 54:], op=AT.add)
                nc.vector.tensor_tensor(out=px_[:, c], in0=px_[:, c],
                                        in1=dpk_t[:, c], op=AT.add)

                # split px = hi*128 + lo so the wrap matmuls can run in bf16
                # (hi <= 211 and lo < 128 are bf16-exact).  px is an exact
                # integer in f32, so the i32 conversion is exact and hi/lo
                # are just a shift and a mask.
                nc.vector.tensor_copy(out=ph_i[:, c], in_=px_[:, c])
                nc.vector.tensor_scalar(out=hi_i[:, c], in0=ph_i[:, c], scalar1=7,
                                        scalar2=None, op0=AT.arith_shift_right)
                nc.vector.tensor_scalar(out=lo_i[:, c], in0=ph_i[:, c], scalar1=127,
                                        scalar2=None, op0=AT.bitwise_and)
                nc.vector.tensor_copy(out=pxhl_[:, c, :54], in_=hi_i[:, c])
                nc.vector.tensor_copy(out=pxhl_[:, c, 54:], in_=lo_i[:, c])

                nc.vector.tensor_tensor(out=l_[:, c], in0=hw_[:, c],
                                        in1=tf_[:, c], op=AT.subtract)
                nc.scalar.activation(out=l1_[:, c], in_=l_[:, c], func=AF.Copy,
                                     scale=-1.0, bias=1.0)

                # corner weights, col-major: wf[p, ci, (pl,k), t] bf16
                nc.vector.tensor_tensor(out=am_[:, c], in0=l1_[:, c, :54],
                                        in1=msk_t[:, c], op=AT.mult)
                nc.vector.tensor_tensor(out=bm_[:, c], in0=l_[:, c, :54],
                                        in1=msk_t[:, c], op=AT.mult)
                for t, (ab, lw0) in enumerate([(am_, l1_), (am_, l_),
                                               (bm_, l1_), (bm_, l_)]):
                    nc.vector.tensor_tensor(out=wf_[:, c, :, t], in0=ab[:, c],
                                            in1=lw0[:, c, 54:], op=AT.mult)

            # ---- wrap: one chunk's px into the dma_gather int16 index
            # layout, materialized directly in all 5 idx bands (parts 0:16 +
            # queue bands 16:32, 48:64, 80:96, 112:128): the banded
            # selection matmuls write wrp[band0+r, q*54+col] = px[q*16+r,
            # col]; one 128-partition DVE copy converts to i16 in the
            # wrapped (col*8+q) order.
            def wrap_chunk(ci):
                wrp = psW.tile([128, 432], F32, tag="wrap", space="PSUM",
                               name=f"wrp_{ci}")
                for q in range(8):
                    # PSUM-accumulated recombine: 128*hi (scaled selection
                    # matrix, cols q*128..) + lo (plain selection, cols
                    # (8+q)*128..)
                    nc.tensor.matmul(out=wrp[:, q * 54:(q + 1) * 54],
                                     lhsT=idf_t[:, q * 128:(q + 1) * 128],
                                     rhs=pxhl_[:, ci, :54], start=True, stop=False)
                    nc.tensor.matmul(out=wrp[:, q * 54:(q + 1) * 54],
                                     lhsT=idf_t[:, (8 + q) * 128:(9 + q) * 128],
                                     rhs=pxhl_[:, ci, 54:], start=False, stop=True)
                nc.vector.tensor_copy(
                    out=wrd_all[:, ci].rearrange("p (col q) -> p q col", q=8),
                    in_=wrp[:].rearrange("p (q col) -> p q col", col=54))

            # segment A: chunks 0-1 ready ASAP; wrap them; the rest of the
            # field phase is emitted just-in-time inside the chunk loop so
            # the scheduler can't starve the early critical chain with it.
            SEGA = 2
            load_seg(0, SEGA)
            field_seg(0, SEGA)
            for ci in range(SEGA):
                wrap_chunk(ci)
            load_seg(SEGA, NCHUNK)
            field_seg(SEGA, NCHUNK)
            if debug:
                nc.sync.dma_start(dbg_px[:], px_[:, 0, :])
                nc.sync.dma_start(dbg_wr[:], wrd_all[16:32, 0, :])
                nc.sync.dma_start(dbg_wf[:], wf_[:, 0, :, :])

            vs_cur = {}
            NG = 4                                     # max chunks per conv group
            for ci in range(NCHUNK):
                # groups: 4x4 then two singles (short pipeline tail)
                gi0 = ci % 4 == 0 or ci >= 16          # group leader chunk
                ng = 4 if ci < 16 else 1               # group size
                c0 = ci - (ci % 4 if ci < 16 else 0)

                # wrap lookahead: keep the PE two chunks ahead of the gathers
                if SEGA <= ci + SEGA < NCHUNK:
                    wrap_chunk(ci + SEGA)

                # ---- gather: one 256B bf16 quad row per (plane, tap, sample)
                G = pg.tile([128, NCOL, ROW], GATHER_DT, tag="G")
                col0 = 0
                if "gather" in skip:
                    nc.vector.memset(G[:, :1, :1], 0)
                for ncols, qn in (CALL_SCHED if "gather" not in skip else []):
                    nidx = ncols * 128
                    gi = nc.gpsimd.dma_gather(
                        G[:, col0:col0 + ncols, :],
                        xq[:],
                        wrd_all[:, ci, col0 * 8: col0 * 8 + nidx // 16],
                        nidx, nidx, ROW, queue_num=qn,
                        single_packet=False)
                    add_dep_helper(gi.ins, lib_inst.ins, sync=False,
                                   reason="mlp library before dma_gather")
                    col0 += ncols

                # ---- corner-weight multiply + corner sum (DVE)
                if gi0:
                    vs_cur[0] = pvs.tile([128, 2, ng, 896], BF16, tag="vs",
                                         name=f"vs_{ci}")
                    nc.vector.memset(vs_cur[0][:, :, :, 864:], 0)
                if "vmul" not in skip:
                    # expand corner weights over c on the (idle) ACT engine so
                    # the DVE multiply gets two contiguous operands; per plane
                    # to halve the buffer
                    V4g = pv.tile([128, NCOL, 4, 32], BF16, tag="v4g", bufs=1)
                    for pl in range(2):
                        sl = slice(pl * K, (pl + 1) * K)
                        wfx = pv.tile([128, K, 4, 32], BF16, tag="wfx",
                                      name=f"wfx_{ci}_{pl}")
                        nc.scalar.activation(
                            out=wfx[:], in_=wf_[:, ci, sl].to_broadcast([128, K, 4, 32]),
                            func=AF.Copy)
                        nc.vector.tensor_tensor(
                            out=V4g[:, sl],
                            in0=G[:, sl].rearrange("p col (t c) -> p col t c", c=32),
                            in1=wfx[:],
                            op=AT.mult)
                    # pairwise corner sums: (t0+t1) + (t2+t3)
                    with nc.allow_low_precision("4-term bf16 corner sum"):
                        t01 = pv.tile([128, NCOL, 32], BF16, tag="t01", bufs=1)
                        nc.vector.tensor_tensor(out=t01[:], in0=V4g[:, :, 0, :],
                                                in1=V4g[:, :, 1, :], op=AT.add)
                        t23 = pv.tile([128, NCOL, 32], BF16, tag="t23", bufs=1)
                        nc.vector.tensor_tensor(out=t23[:], in0=V4g[:, :, 2, :],
                                                in1=V4g[:, :, 3, :], op=AT.add)
                        for pl in range(2):
                            sl = slice(pl * K, (pl + 1) * K)
                            nc.vector.tensor_tensor(
                                out=vs_cur[0][:, pl, ci - c0, :864],
                                in0=t01[:, sl], in1=t23[:, sl], op=AT.add)

                # ---- group end: XBAR transpose + conv matmuls
                if ci - c0 == ng - 1 and "conv" not in skip:
                    r0g = c0 * 128
                    Vs = vs_cur[0]
                    VtT = ptt.tile([128, 2 * ng * 7, 128], BF16, tag="vt",
                                   name=f"vt_{ci}")
                    nc.sync.dma_start_transpose(
                        VtT[:], Vs[:].rearrange("p a b c -> p (a b c)"))
                    rhs4 = VtT[:].rearrange("p (pl c4 g) s -> p pl g c4 s", pl=2, g=7)
                    for pl in range(2):
                        cp = psC.tile([64, ng * 128], F32, tag="conv", space="PSUM",
                                      name=f"cp_{ci}_{pl}")
                        for g in range(7):
                            nc.tensor.matmul(out=cp[:, :ng * 128],
                                             lhsT=wt_t[:, g * 64:(g + 1) * 64],
                                             rhs=rhs4[:, pl, g, :ng],
                                             start=(g == 0), stop=(g == 6))
                        ou = po.tile([64, ng * 128], F32, tag="ou",
                                     name=f"ou_{ci}_{pl}")
                        nc.vector.tensor_scalar(out=ou[:, :ng * 128], in0=cp[:, :ng * 128],
                                                scalar1=bia_t[:64, :],
                                                scalar2=None, op0=AT.add)
                        nc.scalar.dma_start(out[pl, :, r0g:r0g + ng * 128], ou[:, :ng * 128])

    nc.compile()
    return nc


def _prep_static():
    """Input-independent constant tensors."""
    yy, xx = np.meshgrid(np.arange(H), np.arange(W), indexing="ij")
    yy = yy.reshape(-1).astype(np.float32)
    xx = xx.reshape(-1).astype(np.float32)
    kd = (np.arange(K) // 9).astype(np.float32)
    kh = ((np.arange(K) // 3) % 3).astype(np.float32)
    kw = (np.arange(K) % 3).astype(np.float32)

    bases = np.zeros((S, 108), np.float32)
    for pl in range(2):
        bases[:, pl * K:(pl + 1) * K] = yy[:, None] + kh[None, :]
        bases[:, 54 + pl * K:54 + (pl + 1) * K] = xx[:, None] + kw[None, :]

    # banded wrap selection: idf[s, q*128 + band0 + r] = w for s = q*16 + r,
    # bands at partitions {0, 16, 48, 80, 112} (CoreSim + 4 SWDGE queues).
    # Blocks 0..7 carry weight 128 (px hi part), blocks 8..15 weight 1 (lo).
    idf = np.zeros((128, 16, 128), np.float32)
    for q in range(8):
        for r in range(16):
            for band0 in (0, 16, 48, 80, 112):
                idf[q * 16 + r, q, band0 + r] = 128.0
                idf[q * 16 + r, 8 + q, band0 + r] = 1.0
    idf = idf.reshape(128, 16 * 128).astype(ml_dtypes.bfloat16)
    return bases, kd, idf


def _prep_weights(weight, bias):
    # wt rows kc = k*32 + c ; wt[kc, o] = weight[o, c, k]
    wk = weight.reshape(COUT, CIN, K)          # [o, c, k]
    wt = np.zeros((896, COUT), np.float32)
    wt[:864] = wk.transpose(2, 1, 0).reshape(864, COUT)   # [k, c, o] -> rows k*32+c
    # pack [7, 128, 64] -> [128, 7*64] for a single contiguous DMA
    wt = wt.reshape(7, 128, COUT).transpose(1, 0, 2).reshape(128, 7 * COUT)
    wt = np.ascontiguousarray(wt).astype(ml_dtypes.bfloat16)
    bia = bias.reshape(64, 1).astype(np.float32)
    return wt, bia


def _prep_quad(x):
    """x [B, C, D, H, W] -> quad [B, PLANE_PX, 128] bfloat16."""
    xp = np.zeros((B, DP, HPAD + 1, WPAD + 1, CIN), np.float32)
    xp[:, 1:1 + D, 1:1 + H, 1:1 + W, :] = x.transpose(0, 2, 3, 4, 1)
    q = np.empty((B, DP, HPAD, WPAD, 4, CIN), np.float32)
    for t, (cy, j) in enumerate([(0, 0), (0, 1), (1, 0), (1, 1)]):
        q[..., t, :] = xp[:, :, cy:cy + HPAD, j:j + WPAD, :]
    q = q.reshape(B, PLANE_PX, ROW)
    if GATHER_DT == BF16:
        q = q.astype(ml_dtypes.bfloat16)
    return q


def make_in_maps(input, offset, mask, weight, bias):
    if "static" not in _CACHE:
        _CACHE["static"] = _prep_static()
    bases, kd, idf = _CACHE["static"]
    wt, bia = _prep_weights(weight, bias)
    quad = _prep_quad(input)

    offr = offset.reshape(B, K, 2, D, S)   # [b, k, comp, z, s]
    mr = mask.reshape(B, K, D, S)

    in_maps = []
    for core in range(N_CORES):
        bidx = core // 4
        z0 = (2 * core) % 8
        offs_c = np.empty((S, 108), np.float32)
        msk_c = np.empty((S, 54), np.float32)
        dpk_c = np.empty((S, 54), np.float32)
        for pl, z in enumerate((z0, z0 + 1)):
            offs_c[:, pl * K:(pl + 1) * K] = offr[bidx, :, 0, z, :].T
            offs_c[:, 54 + pl * K:54 + (pl + 1) * K] = offr[bidx, :, 1, z, :].T
            msk_c[:, pl * K:(pl + 1) * K] = mr[bidx, :, z, :].T
            dpk_c[:, pl * K:(pl + 1) * K] = ((z + kd) * (HPAD * WPAD))[None, :]
        in_maps.append({
            "xq": quad[bidx],
            "offs": offs_c,
            "msk": msk_c,
            "bases": bases,
            "dpk": dpk_c,
            "wt": wt,
            "bia": bia,
            "idf": idf,
        })
    return in_maps


def kernel(input, offset, mask, weight, bias):
    input = np.ascontiguousarray(input, np.float32)
    offset = np.ascontiguousarray(offset, np.float32)
    mask = np.ascontiguousarray(mask, np.float32)
    weight = np.ascontiguousarray(weight, np.float32)
    bias = np.ascontiguousarray(bias, np.float32)

    if "nc" not in _CACHE:
        _CACHE["nc"] = build_nc()
    nc = _CACHE["nc"]
    in_maps = make_in_maps(input, offset, mask, weight, bias)

    res = run_bass_kernel_spmd(nc, in_maps, core_ids=list(range(N_CORES)))

    out = np.empty((B, COUT, D, H, W), np.float32)
    for core in range(N_CORES):
        bidx = core // 4
        z0 = (2 * core) % 8
        o = np.asarray(res.results[core]["out"], np.float32)   # [2, 64, S]
        out[bidx, :, z0] = o[0].reshape(COUT, H, W)
        out[bidx, :, z0 + 1] = o[1].reshape(COUT, H, W)
    return out



# revision 26
# speedup vs baseline: 1.1212x; 1.0194x over previous
"""Deformable 3D convolution (DeformConv3d) on 8 TRN2 NeuronCores via Bass/Tile.

Strategy (data-parallel over the 16 (b, z) output planes, 2 per core):
  - Host packs x into a zero-padded bf16 "quad image": for every padded pixel
    (dp, hp, wp) a 128-element row [t=(cy,j) major, c minor] holding the
    2x2 bilinear corner patch across all 32 channels.  One dma_gather
    descriptor (256B) fetches all 4 corners x 32 channels for one
    (tap, sample) pair.
  - Device, per core: field phase computes floor/frac/corner weights for all
    18 chunks in a handful of large DVE ops; an upfront wrap phase turns px
    (split hi*128+lo so the selection matmuls run in bf16, recombined by PSUM
    accumulation) into the int16 gather-index layout, materialized directly
    in all SWDGE queue idx bands by banded selection matmuls; per 128-sample
    chunk dma_gather (8 calls round-robin over the 4 queues, queue loads
    alternated per chunk for balance) lands G[s, (pl,k), (t,c)] bf16; the
    corner weights are c-expanded on the ACT engine so the DVE multiply gets
    two contiguous bf16 operands, pairwise adds sum the 4 corners; one XBAR
    DMA-transpose per 4-chunk group flips both planes' weighted sums into
    [kc, s] layout (no PE data transposes); the conv is 7 accumulating bf16
    matmuls of 512 columns per (plane, group), then bias-add and store.

  Bottleneck note: steady state is paced by SWDGE descriptor generation
    (~8.5 ns/descriptor per queue Q7 core, ~0.76 us/call, 4 queues, 1024
    idx/call hardware cap) at ~15-16 us per 6912-descriptor chunk; compute,
    DMA transfer and the conv are all overlapped underneath it.
"""

import numpy as np
import ml_dtypes

import concourse.bass as bass
import concourse.bacc as bacc
import concourse.mybir as mybir
from concourse import tile
from concourse import library_config
from concourse.bass_utils import run_bass_kernel_spmd
from concourse.tile_rust import add_dep_helper

F32 = mybir.dt.float32
BF16 = mybir.dt.bfloat16
I32 = mybir.dt.int32
I16 = mybir.dt.int16
AT = mybir.AluOpType
AF = mybir.ActivationFunctionType
AX = mybir.AxisListType

# problem constants
B, CIN, D, H, W = 2, 32, 8, 48, 48
K, COUT = 27, 64
S = H * W                      # 2304 samples per plane
DP, HPAD, WPAD = 10, 52, 52    # padded depth/rows/cols
PLANE_PX = DP * HPAD * WPAD    # 27040 quad rows per batch
ROW = 128                      # quad row payload elems (4 corners x 32 ch)
NCHUNK = S // 128              # 18
NCOL = 2 * K                   # 54 = (plane, tap) columns per chunk
# dma_gather call splits (<=1024 idx each).  Queue 0's descriptor
# generation runs synchronously ON the Pool engine (~10.6 ns/idx of engine
# residency, observed on HW); queues 1-3 hand off asynchronously (~600 ns
# dispatch) and generate in the background.  So: queues 1-3 carry most of
# the load (dispatched first), queue 0 a small tail share (dispatched
# last, so its engine-blocking overlaps the async queues' background
# generation).
CALL_SCHED = [(7, 1), (7, 2), (7, 3), (6, 0), (7, 1), (7, 2), (7, 3), (6, 0)]
CALL_COLS = [c for c, _ in CALL_SCHED]
N_CORES = 8
NQ = 4

_CACHE = {}
GATHER_DT = BF16               # quad image + G dtype


def build_nc(skip=(), debug=False):
    nc = bacc.Bacc("TRN2", target_bir_lowering=False, debug=False,
                   num_swdge_queues=NQ)
    xq = nc.dram_tensor("xq", [PLANE_PX, ROW], GATHER_DT, kind="ExternalInput")
    offs = nc.dram_tensor("offs", [S, 108], F32, kind="ExternalInput")
    msk = nc.dram_tensor("msk", [S, 54], F32, kind="ExternalInput")
    bases = nc.dram_tensor("bases", [S, 108], F32, kind="ExternalInput")
    dpk = nc.dram_tensor("dpk", [S, 54], F32, kind="ExternalInput")
    wt = nc.dram_tensor("wt", [128, 7 * 64], BF16, kind="ExternalInput")
    bia = nc.dram_tensor("bia", [64, 1], F32, kind="ExternalInput")
    idf = nc.dram_tensor("idf", [128, 16 * 128], BF16, kind="ExternalInput")
    out = nc.dram_tensor("out", [2, 64, S], F32, kind="ExternalOutput")
    if debug:
        dbg_px = nc.dram_tensor("dbg_px", [128, 54], F32, kind="ExternalOutput")
        dbg_wr = nc.dram_tensor("dbg_wr", [16, 432], I16, kind="ExternalOutput")
        dbg_wf = nc.dram_tensor("dbg_wf", [128, 4, 54], BF16, kind="ExternalOutput")

    with tile.TileContext(nc) as tc:
        with (
            tc.tile_pool(name="const", bufs=1) as pc,
            tc.tile_pool(name="fldT", bufs=1) as pt,   # transient field tensors
            tc.tile_pool(name="fldP", bufs=1) as pf,   # persistent px / wf
            tc.tile_pool(name="gg", bufs=3) as pg,
            tc.tile_pool(name="v4", bufs=2) as pv,
            tc.tile_pool(name="vs", bufs=2) as pvs,
            tc.tile_pool(name="vt", bufs=2) as ptt,
            tc.tile_pool(name="oo", bufs=2) as po,
            tc.tile_pool(name="psW", bufs=6, space="PSUM") as psW,
            tc.tile_pool(name="psC", bufs=2, space="PSUM") as psC,
        ):
            wt_t = pc.tile([128, 7 * 64], BF16)
            nc.sync.dma_start(wt_t[:], wt[:])
            bia_t = pc.tile([64, 1], F32)
            nc.sync.dma_start(bia_t[:], bia[:])
            idf_t = pc.tile([128, 16 * 128], BF16)
            nc.sync.dma_start(idf_t[:], idf[:])
            lib_inst = nc.gpsimd.load_library(library_config.mlp)

            # ---- input loads + field phase, in two segments so chunk 0's
            # gathers can dispatch long before the full field phase ends.
            offs_t = pt.tile([128, NCHUNK, 108], F32, tag="offs")
            bases_t = pt.tile([128, NCHUNK, 108], F32, tag="bases")
            msk_t = pt.tile([128, NCHUNK, 54], F32, tag="msk")
            dpk_t = pt.tile([128, NCHUNK, 54], F32, tag="dpk")
            hw_ = pt.tile([128, NCHUNK, 108], F32, tag="hw")
            ti_ = pt.tile([128, NCHUNK, 108], I32, tag="offs", name="ti_")
            tf_ = pt.tile([128, NCHUNK, 108], F32, tag="bases", name="tf_")
            gt_ = pt.tile([128, NCHUNK, 108], F32, tag="gt")
            px_ = pt.tile([128, NCHUNK, 54], F32, tag="bm", name="px_")
            ph_i = pt.tile([128, NCHUNK, 54], I32, tag="offs", name="ph_i")
            hi_i = pt.tile([128, NCHUNK, 54], I32, tag="gt", name="hi_i")
            lo_i = pt.tile([128, NCHUNK, 54], I32, tag="dpk", name="lo_i")
            pxhl_ = pf.tile([128, NCHUNK, 108], BF16, tag="pxhl")
            l_ = pt.tile([128, NCHUNK, 108], F32, tag="gt", name="l_")
            l1_ = pt.tile([128, NCHUNK, 108], F32, tag="hw", name="l1_")
            am_ = pt.tile([128, NCHUNK, 54], F32, tag="dpk", name="am_")
            bm_ = pt.tile([128, NCHUNK, 54], F32, tag="bm")
            wf_ = pf.tile([128, NCHUNK, 54, 4], BF16, tag="wf")
            wrd_all = pf.tile([128, NCHUNK, 432], I16, tag="wrd")

            def load_seg(c0, c1):
                s0, s1 = c0 * 128, c1 * 128
                nch = c1 - c0
                nc.sync.dma_start(
                    offs_t[:, c0:c1],
                    offs[s0:s1].rearrange("(c p) f -> p c f", p=128))
                nc.scalar.dma_start(
                    bases_t[:, c0:c1],
                    bases[s0:s1].rearrange("(c p) f -> p c f", p=128))
                nc.scalar.dma_start(
                    msk_t[:, c0:c1],
                    msk[s0:s1].rearrange("(c p) f -> p c f", p=128))
                nc.sync.dma_start(
                    dpk_t[:, c0:c1],
                    dpk[s0:s1].rearrange("(c p) f -> p c f", p=128))

            def field_seg(c0, c1):
                c = slice(c0, c1)
                nc.vector.tensor_tensor(out=hw_[:, c], in0=offs_t[:, c],
                                        in1=bases_t[:, c], op=AT.add)
                nc.vector.tensor_scalar(out=hw_[:, c], in0=hw_[:, c], scalar1=49.0,
                                        scalar2=0.0, op0=AT.min, op1=AT.max)
                nc.vector.tensor_copy(out=ti_[:, c], in_=hw_[:, c])
                nc.scalar.activation(out=tf_[:, c], in_=ti_[:, c], func=AF.Copy)
                nc.vector.tensor_tensor(out=gt_[:, c], in0=tf_[:, c],
                                        in1=hw_[:, c], op=AT.is_gt)
                nc.vector.tensor_tensor(out=tf_[:, c], in0=tf_[:, c],
                                        in1=gt_[:, c], op=AT.subtract)

                # px = floor_h * 52 + floor_w + dpk  (exact small ints in f32)
                nc.vector.tensor_scalar(out=px_[:, c], in0=tf_[:, c, :54],
                                        scalar1=52.0, scalar2=None, op0=AT.mult)
                nc.vector.tensor_tensor(out=px_[:, c], in0=px_[:, c],
                                        in1=tf_[:, c, 54:], op=AT.add)
                nc.vector.tensor_tensor(out=px_[:, c], in0=px_[:, c],
                                        in1=dpk_t[:, c], op=AT.add)

                # split px = hi*128 + lo so the wrap matmuls can run in bf16
                # (hi <= 211 and lo < 128 are bf16-exact).  px is an exact
                # integer in f32, so the i32 conversion is exact and hi/lo
                # are just a shift and a mask.
                nc.vector.tensor_copy(out=ph_i[:, c], in_=px_[:, c])
                nc.vector.tensor_scalar(out=hi_i[:, c], in0=ph_i[:, c], scalar1=7,
                                        scalar2=None, op0=AT.arith_shift_right)
                nc.vector.tensor_scalar(out=lo_i[:, c], in0=ph_i[:, c], scalar1=127,
                                        scalar2=None, op0=AT.bitwise_and)
                nc.vector.tensor_copy(out=pxhl_[:, c, :54], in_=hi_i[:, c])
                nc.vector.tensor_copy(out=pxhl_[:, c, 54:], in_=lo_i[:, c])

                nc.vector.tensor_tensor(out=l_[:, c], in0=hw_[:, c],
                                        in1=tf_[:, c], op=AT.subtract)
                nc.scalar.activation(out=l1_[:, c], in_=l_[:, c], func=AF.Copy,
                                     scale=-1.0, bias=1.0)

                # corner weights, col-major: wf[p, ci, (pl,k), t] bf16
                nc.vector.tensor_tensor(out=am_[:, c], in0=l1_[:, c, :54],
                                        in1=msk_t[:, c], op=AT.mult)
                nc.vector.tensor_tensor(out=bm_[:, c], in0=l_[:, c, :54],
                                        in1=msk_t[:, c], op=AT.mult)
                for t, (ab, lw0) in enumerate([(am_, l1_), (am_, l_),
                                               (bm_, l1_), (bm_, l_)]):
                    nc.vector.tensor_tensor(out=wf_[:, c, :, t], in0=ab[:, c],
                                            in1=lw0[:, c, 54:], op=AT.mult)

            # ---- wrap: one chunk's px into the dma_gather int16 index
            # layout, materialized directly in all 5 idx bands (parts 0:16 +
            # queue bands 16:32, 48:64, 80:96, 112:128): the banded
            # selection matmuls write wrp[band0+r, q*54+col] = px[q*16+r,
            # col]; one 128-partition DVE copy converts to i16 in the
            # wrapped (col*8+q) order.
            def wrap_chunk(ci):
                wrp = psW.tile([128, 432], F32, tag="wrap", space="PSUM",
                               name=f"wrp_{ci}")
                for q in range(8):
                    # PSUM-accumulated recombine: 128*hi (scaled selection
                    # matrix, cols q*128..) + lo (plain selection, cols
                    # (8+q)*128..)
                    nc.tensor.matmul(out=wrp[:, q * 54:(q + 1) * 54],
                                     lhsT=idf_t[:, q * 128:(q + 1) * 128],
                                     rhs=pxhl_[:, ci, :54], start=True, stop=False)
                    nc.tensor.matmul(out=wrp[:, q * 54:(q + 1) * 54],
                                     lhsT=idf_t[:, (8 + q) * 128:(9 + q) * 128],
                                     rhs=pxhl_[:, ci, 54:], start=False, stop=True)
                nc.vector.tensor_copy(
                    out=wrd_all[:, ci].rearrange("p (col q) -> p q col", q=8),
                    in_=wrp[:].rearrange("p (q col) -> p q col", col=54))

            # segment A: chunks 0-1 ready ASAP; wrap them; the rest of the
            # field phase is emitted just-in-time inside the chunk loop so
            # the scheduler can't starve the early critical chain with it.
            SEGA = 1
            load_seg(0, SEGA)
            field_seg(0, SEGA)
            for ci in range(SEGA):
                wrap_chunk(ci)
            load_seg(SEGA, NCHUNK)
            field_seg(SEGA, NCHUNK)
            if debug:
                nc.sync.dma_start(dbg_px[:], px_[:, 0, :])
                nc.sync.dma_start(dbg_wr[:], wrd_all[16:32, 0, :])
                nc.sync.dma_start(dbg_wf[:], wf_[:, 0, :, :])

            vs_cur = {}
            NG = 4                                     # max chunks per conv group
            for ci in range(NCHUNK):
                # groups: 4x4 then two singles (short pipeline tail)
                gi0 = ci % 4 == 0 or ci >= 16          # group leader chunk
                ng = 4 if ci < 16 else 1               # group size
                c0 = ci - (ci % 4 if ci < 16 else 0)

                # wrap lookahead: keep the PE two chunks ahead of the gathers
                if SEGA <= ci + SEGA < NCHUNK:
                    wrap_chunk(ci + SEGA)

                # ---- gather: one 256B bf16 quad row per (plane, tap, sample)
                G = pg.tile([128, NCOL, ROW], GATHER_DT, tag="G")
                col0 = 0
                if "gather" in skip:
                    nc.vector.memset(G[:, :1, :1], 0)
                for ncols, qn in (CALL_SCHED if "gather" not in skip else []):
                    nidx = ncols * 128
                    gi = nc.gpsimd.dma_gather(
                        G[:, col0:col0 + ncols, :],
                        xq[:],
                        wrd_all[:, ci, col0 * 8: col0 * 8 + nidx // 16],
                        nidx, nidx, ROW, queue_num=qn,
                        single_packet=False)
                    add_dep_helper(gi.ins, lib_inst.ins, sync=False,
                                   reason="mlp library before dma_gather")
                    col0 += ncols

                # ---- corner-weight multiply + corner sum (DVE)
                if gi0:
                    vs_cur[0] = pvs.tile([128, 2, ng, 896], BF16, tag="vs",
                                         name=f"vs_{ci}")
                    nc.vector.memset(vs_cur[0][:, :, :, 864:], 0)
                if "vmul" not in skip:
                    # expand corner weights over c on the (idle) ACT engine so
                    # the DVE multiply gets two contiguous operands; per plane
                    # to halve the buffer
                    V4g = pv.tile([128, NCOL, 4, 32], BF16, tag="v4g", bufs=1)
                    for pl in range(2):
                        sl = slice(pl * K, (pl + 1) * K)
                        wfx = pv.tile([128, K, 4, 32], BF16, tag="wfx",
                                      name=f"wfx_{ci}_{pl}")
                        nc.scalar.activation(
                            out=wfx[:], in_=wf_[:, ci, sl].to_broadcast([128, K, 4, 32]),
                            func=AF.Copy)
                        nc.vector.tensor_tensor(
                            out=V4g[:, sl],
                            in0=G[:, sl].rearrange("p col (t c) -> p col t c", c=32),
                            in1=wfx[:],
                            op=AT.mult)
                    # pairwise corner sums: (t0+t1) + (t2+t3)
                    with nc.allow_low_precision("4-term bf16 corner sum"):
                        t01 = pv.tile([128, NCOL, 32], BF16, tag="t01", bufs=1)
                        nc.vector.tensor_tensor(out=t01[:], in0=V4g[:, :, 0, :],
                                                in1=V4g[:, :, 1, :], op=AT.add)
                        t23 = pv.tile([128, NCOL, 32], BF16, tag="t23", bufs=1)
                        nc.vector.tensor_tensor(out=t23[:], in0=V4g[:, :, 2, :],
                                                in1=V4g[:, :, 3, :], op=AT.add)
                        for pl in range(2):
                            sl = slice(pl * K, (pl + 1) * K)
                            nc.vector.tensor_tensor(
                                out=vs_cur[0][:, pl, ci - c0, :864],
                                in0=t01[:, sl], in1=t23[:, sl], op=AT.add)

                # ---- group end: XBAR transpose + conv matmuls
                if ci - c0 == ng - 1 and "conv" not in skip:
                    r0g = c0 * 128
                    Vs = vs_cur[0]
                    VtT = ptt.tile([128, 2 * ng * 7, 128], BF16, tag="vt",
                                   name=f"vt_{ci}")
                    nc.sync.dma_start_transpose(
                        VtT[:], Vs[:].rearrange("p a b c -> p (a b c)"))
                    rhs4 = VtT[:].rearrange("p (pl c4 g) s -> p pl g c4 s", pl=2, g=7)
                    for pl in range(2):
                        cp = psC.tile([64, ng * 128], F32, tag="conv", space="PSUM",
                                      name=f"cp_{ci}_{pl}")
                        for g in range(7):
                            nc.tensor.matmul(out=cp[:, :ng * 128],
                                             lhsT=wt_t[:, g * 64:(g + 1) * 64],
                                             rhs=rhs4[:, pl, g, :ng],
                                             start=(g == 0), stop=(g == 6))
                        ou = po.tile([64, ng * 128], F32, tag="ou",
                                     name=f"ou_{ci}_{pl}")
                        nc.vector.tensor_scalar(out=ou[:, :ng * 128], in0=cp[:, :ng * 128],
                                                scalar1=bia_t[:64, :],
                                                scalar2=None, op0=AT.add)
                        nc.scalar.dma_start(out[pl, :, r0g:r0g + ng * 128], ou[:, :ng * 128])

    nc.compile()
    return nc


def _prep_static():
    """Input-independent constant tensors."""
    yy, xx = np.meshgrid(np.arange(H), np.arange(W), indexing="ij")
    yy = yy.reshape(-1).astype(np.float32)
    xx = xx.reshape(-1).astype(np.float32)
    kd = (np.arange(K) // 9).astype(np.float32)
    kh = ((np.arange(K) // 3) % 3).astype(np.float32)
    kw = (np.arange(K) % 3).astype(np.float32)

    bases = np.zeros((S, 108), np.float32)
    for pl in range(2):
        bases[:, pl * K:(pl + 1) * K] = yy[:, None] + kh[None, :]
        bases[:, 54 + pl * K:54 + (pl + 1) * K] = xx[:, None] + kw[None, :]

    # banded wrap selection: idf[s, q*128 + band0 + r] = w for s = q*16 + r,
    # bands at partitions {0, 16, 48, 80, 112} (CoreSim + 4 SWDGE queues).
    # Blocks 0..7 carry weight 128 (px hi part), blocks 8..15 weight 1 (lo).
    idf = np.zeros((128, 16, 128), np.float32)
    for q in range(8):
        for r in range(16):
            for band0 in (0, 16, 48, 80, 112):
                idf[q * 16 + r, q, band0 + r] = 128.0
                idf[q * 16 + r, 8 + q, band0 + r] = 1.0
    idf = idf.reshape(128, 16 * 128).astype(ml_dtypes.bfloat16)
    return bases, kd, idf


def _prep_weights(weight, bias):
    # wt rows kc = k*32 + c ; wt[kc, o] = weight[o, c, k]
    wk = weight.reshape(COUT, CIN, K)          # [o, c, k]
    wt = np.zeros((896, COUT), np.float32)
    wt[:864] = wk.transpose(2, 1, 0).reshape(864, COUT)   # [k, c, o] -> rows k*32+c
    # pack [7, 128, 64] -> [128, 7*64] for a single contiguous DMA
    wt = wt.reshape(7, 128, COUT).transpose(1, 0, 2).reshape(128, 7 * COUT)
    wt = np.ascontiguousarray(wt).astype(ml_dtypes.bfloat16)
    bia = bias.reshape(64, 1).astype(np.float32)
    return wt, bia


def _prep_quad(x):
    """x [B, C, D, H, W] -> quad [B, PLANE_PX, 128] bfloat16."""
    xp = np.zeros((B, DP, HPAD + 1, WPAD + 1, CIN), np.float32)
    xp[:, 1:1 + D, 1:1 + H, 1:1 + W, :] = x.transpose(0, 2, 3, 4, 1)
    q = np.empty((B, DP, HPAD, WPAD, 4, CIN), np.float32)
    for t, (cy, j) in enumerate([(0, 0), (0, 1), (1, 0), (1, 1)]):
        q[..., t, :] = xp[:, :, cy:cy + HPAD, j:j + WPAD, :]
    q = q.reshape(B, PLANE_PX, ROW)
    if GATHER_DT == BF16:
        q = q.astype(ml_dtypes.bfloat16)
    return q


def make_in_maps(input, offset, mask, weight, bias):
    if "static" not in _CACHE:
        _CACHE["static"] = _prep_static()
    bases, kd, idf = _CACHE["static"]
    wt, bia = _prep_weights(weight, bias)
    quad = _prep_quad(input)

    offr = offset.reshape(B, K, 2, D, S)   # [b, k, comp, z, s]
    mr = mask.reshape(B, K, D, S)

    in_maps = []
    for core in range(N_CORES):
        bidx = core // 4
        z0 = (2 * core) % 8
        offs_c = np.empty((S, 108), np.float32)
        msk_c = np.empty((S, 54), np.float32)
        dpk_c = np.empty((S, 54), np.float32)
        for pl, z in enumerate((z0, z0 + 1)):
            offs_c[:, pl * K:(pl + 1) * K] = offr[bidx, :, 0, z, :].T
            offs_c[:, 54 + pl * K:54 + (pl + 1) * K] = offr[bidx, :, 1, z, :].T
            msk_c[:, pl * K:(pl + 1) * K] = mr[bidx, :, z, :].T
            dpk_c[:, pl * K:(pl + 1) * K] = ((z + kd) * (HPAD * WPAD))[None, :]
        in_maps.append({
            "xq": quad[bidx],
            "offs": offs_c,
            "msk": msk_c,
            "bases": bases,
            "dpk": dpk_c,
            "wt": wt,
            "bia": bia,
            "idf": idf,
        })
    return in_maps


def kernel(input, offset, mask, weight, bias):
    input = np.ascontiguousarray(input, np.float32)
    offset = np.ascontiguousarray(offset, np.float32)
    mask = np.ascontiguousarray(mask, np.float32)
    weight = np.ascontiguousarray(weight, np.float32)
    bias = np.ascontiguousarray(bias, np.float32)

    if "nc" not in _CACHE:
        _CACHE["nc"] = build_nc()
    nc = _CACHE["nc"]
    in_maps = make_in_maps(input, offset, mask, weight, bias)

    res = run_bass_kernel_spmd(nc, in_maps, core_ids=list(range(N_CORES)))

    out = np.empty((B, COUT, D, H, W), np.float32)
    for core in range(N_CORES):
        bidx = core // 4
        z0 = (2 * core) % 8
        o = np.asarray(res.results[core]["out"], np.float32)   # [2, 64, S]
        out[bidx, :, z0] = o[0].reshape(COUT, H, W)
        out[bidx, :, z0 + 1] = o[1].reshape(COUT, H, W)
    return out



# revision 27
# speedup vs baseline: 1.1231x; 1.0017x over previous
"""Deformable 3D convolution (DeformConv3d) on 8 TRN2 NeuronCores via Bass/Tile.

Strategy (data-parallel over the 16 (b, z) output planes, 2 per core):
  - Host packs x into a zero-padded bf16 "quad image": for every padded pixel
    (dp, hp, wp) a 128-element row [t=(cy,j) major, c minor] holding the
    2x2 bilinear corner patch across all 32 channels.  One dma_gather
    descriptor (256B) fetches all 4 corners x 32 channels for one
    (tap, sample) pair.
  - Device, per core: the field phase (floor/frac/corner weights) runs in two
    segments — chunk 0 first so its gathers can dispatch at ~40 us, the other
    17 chunks in one fused set of large DVE ops; the wrap phase turns px
    (split hi*128+lo so the selection matmuls run in bf16, recombined by PSUM
    accumulation) into the int16 gather-index layout, materialized directly
    in all SWDGE queue idx bands by banded selection matmuls, emitted with a
    2-chunk lookahead inside the main loop so the PE stays just ahead of the
    gathers; per 128-sample chunk dma_gather lands G[s, (pl,k), (t,c)] bf16;
    the corner weights are c-expanded on the ACT engine so the DVE multiply
    gets two contiguous bf16 operands, pairwise adds sum the 4 corners; one
    XBAR DMA-transpose per conv group (4+4+4+4+1+1 chunks — single-chunk
    tail groups shorten the pipeline drain) flips both planes' weighted sums
    into [kc, s] layout; the conv is 7 accumulating bf16 matmuls per
    (plane, group), then bias-add and store.

  Gather scheduling (measured on HW): a dma_gather on queue 0 occupies the
    Pool engine for its whole descriptor generation (~10.6 ns/idx of engine
    residency), while queues 1-3 dispatch in ~600 ns and generate in the
    background at a similar per-queue rate.  So queues 1-3 carry 14 of the
    54 (plane,tap) columns each and queue 0 carries 12, with queue 0's two
    calls dispatched after a 3-call async batch each, so its engine-blocking
    overlaps the async queues' background generation.  Per-queue descriptor
    generation (~8.5-10.5 ns/idx sustained) remains the pacer: ~15.5 us per
    6912-descriptor chunk; field/wrap compute, DMA transfer and the conv
    all overlap underneath it.
"""

import numpy as np
import ml_dtypes

import concourse.bass as bass
import concourse.bacc as bacc
import concourse.mybir as mybir
from concourse import tile
from concourse import library_config
from concourse.bass_utils import run_bass_kernel_spmd
from concourse.tile_rust import add_dep_helper

F32 = mybir.dt.float32
BF16 = mybir.dt.bfloat16
I32 = mybir.dt.int32
I16 = mybir.dt.int16
AT = mybir.AluOpType
AF = mybir.ActivationFunctionType
AX = mybir.AxisListType

# problem constants
B, CIN, D, H, W = 2, 32, 8, 48, 48
K, COUT = 27, 64
S = H * W                      # 2304 samples per plane
DP, HPAD, WPAD = 10, 52, 52    # padded depth/rows/cols
PLANE_PX = DP * HPAD * WPAD    # 27040 quad rows per batch
ROW = 128                      # quad row payload elems (4 corners x 32 ch)
NCHUNK = S // 128              # 18
NCOL = 2 * K                   # 54 = (plane, tap) columns per chunk
# dma_gather call splits (<=1024 idx each).  Queue 0's descriptor
# generation runs synchronously ON the Pool engine (~10.6 ns/idx of engine
# residency, observed on HW); queues 1-3 hand off asynchronously (~600 ns
# dispatch) and generate in the background.  So: queues 1-3 carry most of
# the load (dispatched first), queue 0 a small tail share (dispatched
# last, so its engine-blocking overlaps the async queues' background
# generation).
CALL_SCHED = [(7, 1), (7, 2), (7, 3), (6, 0), (7, 1), (7, 2), (7, 3), (6, 0)]
CALL_COLS = [c for c, _ in CALL_SCHED]
N_CORES = 8
NQ = 4

_CACHE = {}
GATHER_DT = BF16               # quad image + G dtype


def build_nc(skip=(), debug=False):
    nc = bacc.Bacc("TRN2", target_bir_lowering=False, debug=False,
                   num_swdge_queues=NQ)
    xq = nc.dram_tensor("xq", [PLANE_PX, ROW], GATHER_DT, kind="ExternalInput")
    offs = nc.dram_tensor("offs", [S, 108], F32, kind="ExternalInput")
    msk = nc.dram_tensor("msk", [S, 54], F32, kind="ExternalInput")
    bases = nc.dram_tensor("bases", [S, 108], F32, kind="ExternalInput")
    dpk = nc.dram_tensor("dpk", [S, 54], F32, kind="ExternalInput")
    wt = nc.dram_tensor("wt", [128, 7 * 64], BF16, kind="ExternalInput")
    bia = nc.dram_tensor("bia", [64, 1], F32, kind="ExternalInput")
    idf = nc.dram_tensor("idf", [128, 16 * 128], BF16, kind="ExternalInput")
    out = nc.dram_tensor("out", [2, 64, S], F32, kind="ExternalOutput")
    if debug:
        dbg_px = nc.dram_tensor("dbg_px", [128, 54], F32, kind="ExternalOutput")
        dbg_wr = nc.dram_tensor("dbg_wr", [16, 432], I16, kind="ExternalOutput")
        dbg_wf = nc.dram_tensor("dbg_wf", [128, 4, 54], BF16, kind="ExternalOutput")

    with tile.TileContext(nc) as tc:
        with (
            tc.tile_pool(name="const", bufs=1) as pc,
            tc.tile_pool(name="fldT", bufs=1) as pt,   # transient field tensors
            tc.tile_pool(name="fldP", bufs=1) as pf,   # persistent px / wf
            tc.tile_pool(name="gg", bufs=3) as pg,
            tc.tile_pool(name="v4", bufs=2) as pv,
            tc.tile_pool(name="vs", bufs=2) as pvs,
            tc.tile_pool(name="vt", bufs=2) as ptt,
            tc.tile_pool(name="oo", bufs=2) as po,
            tc.tile_pool(name="psW", bufs=6, space="PSUM") as psW,
            tc.tile_pool(name="psC", bufs=2, space="PSUM") as psC,
        ):
            wt_t = pc.tile([128, 7 * 64], BF16)
            nc.sync.dma_start(wt_t[:], wt[:])
            bia_t = pc.tile([64, 1], F32)
            nc.sync.dma_start(bia_t[:], bia[:])
            idf_t = pc.tile([128, 16 * 128], BF16)
            nc.sync.dma_start(idf_t[:], idf[:])
            lib_inst = nc.gpsimd.load_library(library_config.mlp)

            # ---- input loads + field phase, in two segments so chunk 0's
            # gathers can dispatch long before the full field phase ends.
            offs_t = pt.tile([128, NCHUNK, 108], F32, tag="offs")
            bases_t = pt.tile([128, NCHUNK, 108], F32, tag="bases")
            msk_t = pt.tile([128, NCHUNK, 54], F32, tag="msk")
            dpk_t = pt.tile([128, NCHUNK, 54], F32, tag="dpk")
            hw_ = pt.tile([128, NCHUNK, 108], F32, tag="hw")
            ti_ = pt.tile([128, NCHUNK, 108], I32, tag="offs", name="ti_")
            tf_ = pt.tile([128, NCHUNK, 108], F32, tag="bases", name="tf_")
            gt_ = pt.tile([128, NCHUNK, 108], F32, tag="gt")
            px_ = pt.tile([128, NCHUNK, 54], F32, tag="bm", name="px_")
            ph_i = pt.tile([128, NCHUNK, 54], I32, tag="offs", name="ph_i")
            hi_i = pt.tile([128, NCHUNK, 54], I32, tag="gt", name="hi_i")
            lo_i = pt.tile([128, NCHUNK, 54], I32, tag="dpk", name="lo_i")
            pxhl_ = pf.tile([128, NCHUNK, 108], BF16, tag="pxhl")
            l_ = pt.tile([128, NCHUNK, 108], F32, tag="gt", name="l_")
            l1_ = pt.tile([128, NCHUNK, 108], F32, tag="hw", name="l1_")
            am_ = pt.tile([128, NCHUNK, 54], F32, tag="dpk", name="am_")
            bm_ = pt.tile([128, NCHUNK, 54], F32, tag="bm")
            wf_ = pf.tile([128, NCHUNK, 54, 4], BF16, tag="wf")
            wrd_all = pf.tile([128, NCHUNK, 432], I16, tag="wrd")

            def load_seg(c0, c1):
                s0, s1 = c0 * 128, c1 * 128
                nch = c1 - c0
                nc.sync.dma_start(
                    offs_t[:, c0:c1],
                    offs[s0:s1].rearrange("(c p) f -> p c f", p=128))
                nc.scalar.dma_start(
                    bases_t[:, c0:c1],
                    bases[s0:s1].rearrange("(c p) f -> p c f", p=128))
                nc.scalar.dma_start(
                    msk_t[:, c0:c1],
                    msk[s0:s1].rearrange("(c p) f -> p c f", p=128))
                nc.sync.dma_start(
                    dpk_t[:, c0:c1],
                    dpk[s0:s1].rearrange("(c p) f -> p c f", p=128))

            def field_seg(c0, c1):
                c = slice(c0, c1)
                nc.vector.tensor_tensor(out=hw_[:, c], in0=offs_t[:, c],
                                        in1=bases_t[:, c], op=AT.add)
                nc.vector.tensor_scalar(out=hw_[:, c], in0=hw_[:, c], scalar1=49.0,
                                        scalar2=0.0, op0=AT.min, op1=AT.max)
                nc.vector.tensor_copy(out=ti_[:, c], in_=hw_[:, c])
                nc.scalar.activation(out=tf_[:, c], in_=ti_[:, c], func=AF.Copy)
                nc.vector.tensor_tensor(out=gt_[:, c], in0=tf_[:, c],
                                        in1=hw_[:, c], op=AT.is_gt)
                nc.vector.tensor_tensor(out=tf_[:, c], in0=tf_[:, c],
                                        in1=gt_[:, c], op=AT.subtract)

                # px = floor_h * 52 + floor_w + dpk  (exact small ints in f32)
                nc.vector.tensor_scalar(out=px_[:, c], in0=tf_[:, c, :54],
                                        scalar1=52.0, scalar2=None, op0=AT.mult)
                nc.vector.tensor_tensor(out=px_[:, c], in0=px_[:, c],
                                        in1=tf_[:, c, 54:], op=AT.add)
                nc.vector.tensor_tensor(out=px_[:, c], in0=px_[:, c],
                                        in1=dpk_t[:, c], op=AT.add)

                # split px = hi*128 + lo so the wrap matmuls can run in bf16
                # (hi <= 211 and lo < 128 are bf16-exact).  px is an exact
                # integer in f32, so the i32 conversion is exact and hi/lo
                # are just a shift and a mask.
                nc.vector.tensor_copy(out=ph_i[:, c], in_=px_[:, c])
                nc.vector.tensor_scalar(out=hi_i[:, c], in0=ph_i[:, c], scalar1=7,
                                        scalar2=None, op0=AT.arith_shift_right)
                nc.vector.tensor_scalar(out=lo_i[:, c], in0=ph_i[:, c], scalar1=127,
                                        scalar2=None, op0=AT.bitwise_and)
                nc.vector.tensor_copy(out=pxhl_[:, c, :54], in_=hi_i[:, c])
                nc.vector.tensor_copy(out=pxhl_[:, c, 54:], in_=lo_i[:, c])

                nc.vector.tensor_tensor(out=l_[:, c], in0=hw_[:, c],
                                        in1=tf_[:, c], op=AT.subtract)
                nc.scalar.activation(out=l1_[:, c], in_=l_[:, c], func=AF.Copy,
                                     scale=-1.0, bias=1.0)

                # corner weights, col-major: wf[p, ci, (pl,k), t] bf16
                nc.vector.tensor_tensor(out=am_[:, c], in0=l1_[:, c, :54],
                                        in1=msk_t[:, c], op=AT.mult)
                nc.vector.tensor_tensor(out=bm_[:, c], in0=l_[:, c, :54],
                                        in1=msk_t[:, c], op=AT.mult)
                for t, (ab, lw0) in enumerate([(am_, l1_), (am_, l_),
                                               (bm_, l1_), (bm_, l_)]):
                    nc.vector.tensor_tensor(out=wf_[:, c, :, t], in0=ab[:, c],
                                            in1=lw0[:, c, 54:], op=AT.mult)

            # ---- wrap: one chunk's px into the dma_gather int16 index
            # layout, materialized directly in all 5 idx bands (parts 0:16 +
            # queue bands 16:32, 48:64, 80:96, 112:128): the banded
            # selection matmuls write wrp[band0+r, q*54+col] = px[q*16+r,
            # col]; one 128-partition DVE copy converts to i16 in the
            # wrapped (col*8+q) order.
            def wrap_chunk(ci):
                wrp = psW.tile([128, 432], F32, tag="wrap", space="PSUM",
                               name=f"wrp_{ci}")
                for q in range(8):
                    # PSUM-accumulated recombine: 128*hi (scaled selection
                    # matrix, cols q*128..) + lo (plain selection, cols
                    # (8+q)*128..)
                    nc.tensor.matmul(out=wrp[:, q * 54:(q + 1) * 54],
                                     lhsT=idf_t[:, q * 128:(q + 1) * 128],
                                     rhs=pxhl_[:, ci, :54], start=True, stop=False)
                    nc.tensor.matmul(out=wrp[:, q * 54:(q + 1) * 54],
                                     lhsT=idf_t[:, (8 + q) * 128:(9 + q) * 128],
                                     rhs=pxhl_[:, ci, 54:], start=False, stop=True)
                nc.vector.tensor_copy(
                    out=wrd_all[:, ci].rearrange("p (col q) -> p q col", q=8),
                    in_=wrp[:].rearrange("p (q col) -> p q col", col=54))

            # segment A: chunks 0-1 ready ASAP; wrap them; the rest of the
            # field phase is emitted just-in-time inside the chunk loop so
            # the scheduler can't starve the early critical chain with it.
            SEGA = 1
            load_seg(0, SEGA)
            field_seg(0, SEGA)
            for ci in range(SEGA):
                wrap_chunk(ci)
            load_seg(SEGA, NCHUNK)
            field_seg(SEGA, NCHUNK)
            if debug:
                nc.sync.dma_start(dbg_px[:], px_[:, 0, :])
                nc.sync.dma_start(dbg_wr[:], wrd_all[16:32, 0, :])
                nc.sync.dma_start(dbg_wf[:], wf_[:, 0, :, :])

            vs_cur = {}
            NG = 4                                     # max chunks per conv group
            for ci in range(NCHUNK):
                # groups: 4x4 then two singles (short pipeline tail)
                gi0 = ci % 4 == 0 or ci >= 16          # group leader chunk
                ng = 4 if ci < 16 else 1               # group size
                c0 = ci - (ci % 4 if ci < 16 else 0)

                # wrap lookahead: keep the PE two chunks ahead of the gathers
                if SEGA <= ci + SEGA < NCHUNK:
                    wrap_chunk(ci + SEGA)

                # ---- gather: one 256B bf16 quad row per (plane, tap, sample)
                G = pg.tile([128, NCOL, ROW], GATHER_DT, tag="G")
                col0 = 0
                if "gather" in skip:
                    nc.vector.memset(G[:, :1, :1], 0)
                for ncols, qn in (CALL_SCHED if "gather" not in skip else []):
                    nidx = ncols * 128
                    gi = nc.gpsimd.dma_gather(
                        G[:, col0:col0 + ncols, :],
                        xq[:],
                        wrd_all[:, ci, col0 * 8: col0 * 8 + nidx // 16],
                        nidx, nidx, ROW, queue_num=qn,
                        single_packet=False)
                    add_dep_helper(gi.ins, lib_inst.ins, sync=False,
                                   reason="mlp library before dma_gather")
                    col0 += ncols

                # ---- corner-weight multiply + corner sum (DVE)
                if gi0:
                    vs_cur[0] = pvs.tile([128, 2, ng, 896], BF16, tag="vs",
                                         name=f"vs_{ci}")
                    nc.vector.memset(vs_cur[0][:, :, :, 864:], 0)
                if "vmul" not in skip:
                    # expand corner weights over c on the (idle) ACT engine so
                    # the DVE multiply gets two contiguous operands; per plane
                    # to halve the buffer
                    V4g = pv.tile([128, NCOL, 4, 32], BF16, tag="v4g", bufs=1)
                    for pl in range(2):
                        sl = slice(pl * K, (pl + 1) * K)
                        wfx = pv.tile([128, K, 4, 32], BF16, tag="wfx",
                                      name=f"wfx_{ci}_{pl}")
                        nc.scalar.activation(
                            out=wfx[:], in_=wf_[:, ci, sl].to_broadcast([128, K, 4, 32]),
                            func=AF.Copy)
                        nc.vector.tensor_tensor(
                            out=V4g[:, sl],
                            in0=G[:, sl].rearrange("p col (t c) -> p col t c", c=32),
                            in1=wfx[:],
                            op=AT.mult)
                    # pairwise corner sums: (t0+t1) + (t2+t3)
                    with nc.allow_low_precision("4-term bf16 corner sum"):
                        t01 = pv.tile([128, NCOL, 32], BF16, tag="t01", bufs=1)
                        nc.vector.tensor_tensor(out=t01[:], in0=V4g[:, :, 0, :],
                                                in1=V4g[:, :, 1, :], op=AT.add)
                        t23 = pv.tile([128, NCOL, 32], BF16, tag="t23", bufs=1)
                        nc.vector.tensor_tensor(out=t23[:], in0=V4g[:, :, 2, :],
                                                in1=V4g[:, :, 3, :], op=AT.add)
                        for pl in range(2):
                            sl = slice(pl * K, (pl + 1) * K)
                            nc.vector.tensor_tensor(
                                out=vs_cur[0][:, pl, ci - c0, :864],
                                in0=t01[:, sl], in1=t23[:, sl], op=AT.add)

                # ---- group end: XBAR transpose + conv matmuls
                if ci - c0 == ng - 1 and "conv" not in skip:
                    r0g = c0 * 128
                    Vs = vs_cur[0]
                    VtT = ptt.tile([128, 2 * ng * 7, 128], BF16, tag="vt",
                                   name=f"vt_{ci}")
                    nc.sync.dma_start_transpose(
                        VtT[:], Vs[:].rearrange("p a b c -> p (a b c)"))
                    rhs4 = VtT[:].rearrange("p (pl c4 g) s -> p pl g c4 s", pl=2, g=7)
                    for pl in range(2):
                        cp = psC.tile([64, ng * 128], F32, tag="conv", space="PSUM",
                                      name=f"cp_{ci}_{pl}")
                        for g in range(7):
                            nc.tensor.matmul(out=cp[:, :ng * 128],
                                             lhsT=wt_t[:, g * 64:(g + 1) * 64],
                                             rhs=rhs4[:, pl, g, :ng],
                                             start=(g == 0), stop=(g == 6))
                        ou = po.tile([64, ng * 128], F32, tag="ou",
                                     name=f"ou_{ci}_{pl}")
                        nc.vector.tensor_scalar(out=ou[:, :ng * 128], in0=cp[:, :ng * 128],
                                                scalar1=bia_t[:64, :],
                                                scalar2=None, op0=AT.add)
                        nc.scalar.dma_start(out[pl, :, r0g:r0g + ng * 128], ou[:, :ng * 128])

    nc.compile()
    return nc


def _prep_static():
    """Input-independent constant tensors."""
    yy, xx = np.meshgrid(np.arange(H), np.arange(W), indexing="ij")
    yy = yy.reshape(-1).astype(np.float32)
    xx = xx.reshape(-1).astype(np.float32)
    kd = (np.arange(K) // 9).astype(np.float32)
    kh = ((np.arange(K) // 3) % 3).astype(np.float32)
    kw = (np.arange(K) % 3).astype(np.float32)

    bases = np.zeros((S, 108), np.float32)
    for pl in range(2):
        bases[:, pl * K:(pl + 1) * K] = yy[:, None] + kh[None, :]
        bases[:, 54 + pl * K:54 + (pl + 1) * K] = xx[:, None] + kw[None, :]

    # banded wrap selection: idf[s, q*128 + band0 + r] = w for s = q*16 + r,
    # bands at partitions {0, 16, 48, 80, 112} (CoreSim + 4 SWDGE queues).
    # Blocks 0..7 carry weight 128 (px hi part), blocks 8..15 weight 1 (lo).
    idf = np.zeros((128, 16, 128), np.float32)
    for q in range(8):
        for r in range(16):
            for band0 in (0, 16, 48, 80, 112):
                idf[q * 16 + r, q, band0 + r] = 128.0
                idf[q * 16 + r, 8 + q, band0 + r] = 1.0
    idf = idf.reshape(128, 16 * 128).astype(ml_dtypes.bfloat16)
    return bases, kd, idf


def _prep_weights(weight, bias):
    # wt rows kc = k*32 + c ; wt[kc, o] = weight[o, c, k]
    wk = weight.reshape(COUT, CIN, K)          # [o, c, k]
    wt = np.zeros((896, COUT), np.float32)
    wt[:864] = wk.transpose(2, 1, 0).reshape(864, COUT)   # [k, c, o] -> rows k*32+c
    # pack [7, 128, 64] -> [128, 7*64] for a single contiguous DMA
    wt = wt.reshape(7, 128, COUT).transpose(1, 0, 2).reshape(128, 7 * COUT)
    wt = np.ascontiguousarray(wt).astype(ml_dtypes.bfloat16)
    bia = bias.reshape(64, 1).astype(np.float32)
    return wt, bia


def _prep_quad(x):
    """x [B, C, D, H, W] -> quad [B, PLANE_PX, 128] bfloat16."""
    xp = np.zeros((B, DP, HPAD + 1, WPAD + 1, CIN), np.float32)
    xp[:, 1:1 + D, 1:1 + H, 1:1 + W, :] = x.transpose(0, 2, 3, 4, 1)
    q = np.empty((B, DP, HPAD, WPAD, 4, CIN), np.float32)
    for t, (cy, j) in enumerate([(0, 0), (0, 1), (1, 0), (1, 1)]):
        q[..., t, :] = xp[:, :, cy:cy + HPAD, j:j + WPAD, :]
    q = q.reshape(B, PLANE_PX, ROW)
    if GATHER_DT == BF16:
        q = q.astype(ml_dtypes.bfloat16)
    return q


def make_in_maps(input, offset, mask, weight, bias):
    if "static" not in _CACHE:
        _CACHE["static"] = _prep_static()
    bases, kd, idf = _CACHE["static"]
    wt, bia = _prep_weights(weight, bias)
    quad = _prep_quad(input)

    offr = offset.reshape(B, K, 2, D, S)   # [b, k, comp, z, s]
    mr = mask.reshape(B, K, D, S)

    in_maps = []
    for core in range(N_CORES):
        bidx = core // 4
        z0 = (2 * core) % 8
        offs_c = np.empty((S, 108), np.float32)
        msk_c = np.empty((S, 54), np.float32)
        dpk_c = np.empty((S, 54), np.float32)
        for pl, z in enumerate((z0, z0 + 1)):
            offs_c[:, pl * K:(pl + 1) * K] = offr[bidx, :, 0, z, :].T
            offs_c[:, 54 + pl * K:54 + (pl + 1) * K] = offr[bidx, :, 1, z, :].T
            msk_c[:, pl * K:(pl + 1) * K] = mr[bidx, :, z, :].T
            dpk_c[:, pl * K:(pl + 1) * K] = ((z + kd) * (HPAD * WPAD))[None, :]
        in_maps.append({
            "xq": quad[bidx],
            "offs": offs_c,
            "msk": msk_c,
            "bases": bases,
            "dpk": dpk_c,
            "wt": wt,
            "bia": bia,
            "idf": idf,
        })
    return in_maps


def kernel(input, offset, mask, weight, bias):
    input = np.ascontiguousarray(input, np.float32)
    offset = np.ascontiguousarray(offset, np.float32)
    mask = np.ascontiguousarray(mask, np.float32)
    weight = np.ascontiguousarray(weight, np.float32)
    bias = np.ascontiguousarray(bias, np.float32)

    if "nc" not in _CACHE:
        _CACHE["nc"] = build_nc()
    nc = _CACHE["nc"]
    in_maps = make_in_maps(input, offset, mask, weight, bias)

    res = run_bass_kernel_spmd(nc, in_maps, core_ids=list(range(N_CORES)))

    out = np.empty((B, COUT, D, H, W), np.float32)
    for core in range(N_CORES):
        bidx = core // 4
        z0 = (2 * core) % 8
        o = np.asarray(res.results[core]["out"], np.float32)   # [2, 64, S]
        out[bidx, :, z0] = o[0].reshape(COUT, H, W)
        out[bidx, :, z0 + 1] = o[1].reshape(COUT, H, W)
    return out

